# revision 5
# baseline (speedup 1.0000x reference)
"""Trainium2 Bass kernel for a 3-layer spiking net (snntorch-style Leaky/LIF).

Math (per timestep t, eval mode):
    cur1 = x_t @ w1.T + b1
    mem1 = 0.9*mem1 + cur1 - (mem1_prev > 1)        # reset-by-subtract
    spk1 = (mem1 > 1)
    cur2 = spk1 @ w2.T + b2
    mem2 = 0.85*mem2 + cur2 - (mem2_prev > 1)
    spk2 = (mem2 > 1)
    out_t = spk2 @ w3.T + b3

Strategy (v2):
  - Data-parallel over batch: B=64 -> 8 cores x 8.
  - Matmuls batched over T in chunks; only the elementwise LIF updates are
    sequential.  The serial chains (2 scalar_tensor_tensor ops per step per
    layer, fp32, bit-exact vs the reference via the negated-membrane trick)
    run on the DVE; NOTHING else runs on the DVE, so the chain streams
    back-to-back (~127ns/op, 1024 ops ~ 130us).
  - Spike extraction (batched is_lt every 16 steps) runs on the Pool
    engine (gpsimd), which is otherwise idle -- it no longer interrupts
    the DVE chain.  spk tiles are fp16 {0,1} (exact).
  - Matmul 1: fp16 3-pass as before (x hi/lo + w1 lo with 2^10 shifts),
    known band ~2.7e-7.  82us of PE.
  - Matmul 2: fp16-hi + fp8-lo: cur2 = w2h^T s + (w2lo8^T s12) where
    w2h = fp16(w2), w2lo8 = e4m3((w2 - w2h) * 2^12) via a DoubleRow
    (2x-rate) fp8 matmul against s12 = e5m2(spk * 2^-12) (exact: powers
    of two).  Effective w2 residual ~2^-21, same as the old bf16 hi/lo
    2-pass, at 10/16 the PE cycles (68us).  Probe-verified exact on HW.
  - Matmul 3: fp16 w3 single pass (output error ~1e-4, was 1.5e-3 bf16).
  - s12 tiles are derived from the fp16 spikes by ACT copies with scale
    2^-12 (exact, probe-verified).
  - Pipeline: MM1(c+1) -> MM2(c) on PE; scan1(c+1) before scan2(c) on
    DVE; MM3 in the epilogue to fill the PE drain window.
"""

import sys

for _p in ("/opt/trn_rl_repo", "/root/.axon_site/_ro/pypackages"):
    if _p not in sys.path:
        sys.path.insert(0, _p)

import ml_dtypes
import numpy as np

import concourse.bass as bass
import concourse.mybir as mybir
from concourse import bacc, tile
from concourse.bass_utils import run_bass_kernel_spmd

F32 = mybir.dt.float32
F16 = mybir.dt.float16
BF16 = mybir.dt.bfloat16
F8E4 = mybir.dt.float8e4
F8E5 = mybir.dt.float8e5
ALU = mybir.AluOpType
ACTF = mybir.ActivationFunctionType
DR = mybir.MatmulPerfMode.DoubleRow

# Problem shape (hardcoded; harness runs kernel.py standalone).
T, B, I, H1, H2, O = 256, 64, 512, 1024, 1024, 256
NCORES = 8
BL = B // NCORES          # batch per core
BETA1, BETA2 = 0.9, 0.85
KI = I // 128             # K-tiles for matmul 1 (4)
J1 = H1 // 128            # M-tiles for layer 1 (8)
J2 = H2 // 128            # M-tiles for layer 2 (8)


def build(n_t=T, sched=None, trace_sim=False, opts=None):
    """Build the per-core SPMD program. Identical on all cores."""
    if sched is None:
        sched = (opts or {}).get("sched") or [16, 48, 48, 48, 32, 32, 16, 16]
    assert sum(sched) == n_t and all(s % 8 == 0 for s in sched)
    nc = bacc.Bacc("TRN2", target_bir_lowering=False, debug=False)

    xh = nc.declare_dram_parameter("xh", [I, n_t * BL], F16, isOutput=False)
    xl = nc.declare_dram_parameter("xl", [I, n_t * BL], F16, isOutput=False)
    xh10 = nc.declare_dram_parameter("xh10", [I, n_t * BL], F16,
                                     isOutput=False)
    w1h = nc.declare_dram_parameter("w1h", [I, H1], F16, isOutput=False)
    w1l10 = nc.declare_dram_parameter("w1l10", [I, H1], F16, isOutput=False)
    w2h = nc.declare_dram_parameter("w2h", [H1, H2], F16, isOutput=False)
    w2lo = nc.declare_dram_parameter("w2lo", [H1, H2], F8E4, isOutput=False)
    w3h = nc.declare_dram_parameter("w3h", [H2, O], F16, isOutput=False)
    b2c = nc.declare_dram_parameter("b2c", [128, J2], F32, isOutput=False)
    rs3h = nc.declare_dram_parameter("rs3h", [1, O], F16, isOutput=False)
    ones16 = nc.declare_dram_parameter("ones16", [1, 128], F16,
                                       isOutput=False)
    y = nc.declare_dram_parameter("y", [n_t * BL, O], F32, isOutput=True)

    with tile.TileContext(nc, trace_sim=trace_sim) as tc_ctx:
        _body(nc, tc_ctx, (xh, xl, xh10), (w1h, w1l10), w2h, w2lo, w3h,
              (b2c, rs3h, ones16), y, n_t, sched, opts or {})
    nc.compile()
    return nc


def _body(nc, tc_ctx, x_ds, w1_ds, w2h_d, w2lo_d, w3h_d, corr_ds, y,
          n_t, sched, opts):
    b2c_d, rs3h_d, ones16_d = corr_ds
    ext = opts.get("ext", "dve")
    ext1 = opts.get("ext1", ext)
    ext2 = opts.get("ext2", ext)
    nch = len(sched)
    ntb0 = sched[0] * BL
    import contextlib

    ctx = contextlib.ExitStack()
    with ctx:
        cb = opts.get("cur_bufs", 2)
        pb1, pb2, pb3 = opts.get("psum_bufs", (3, 3, 2))
        wsb = ctx.enter_context(tc_ctx.tile_pool(name="wsb", bufs=1))
        xt_pool = ctx.enter_context(
            tc_ctx.tile_pool(name="xt", bufs=opts.get("xt_bufs", 2)))
        cur1_pool = ctx.enter_context(tc_ctx.tile_pool(name="cur1", bufs=cb))
        spk1_pool = ctx.enter_context(tc_ctx.tile_pool(name="spk1", bufs=2))
        s12_pool = ctx.enter_context(tc_ctx.tile_pool(name="s12", bufs=2))
        cur2_pool = ctx.enter_context(tc_ctx.tile_pool(name="cur2", bufs=cb))
        spk2_pool = ctx.enter_context(tc_ctx.tile_pool(name="spk2",
                                                       bufs=len(sched)))
        out_pool = ctx.enter_context(tc_ctx.tile_pool(name="outp", bufs=3))
        pp1 = ctx.enter_context(tc_ctx.tile_pool(name="pp1", bufs=pb1,
                                                 space="PSUM"))
        pp2 = ctx.enter_context(tc_ctx.tile_pool(name="pp2", bufs=pb2,
                                                 space="PSUM"))
        pp3 = ctx.enter_context(tc_ctx.tile_pool(name="pp3", bufs=pb3,
                                                 space="PSUM"))

        # ---- weight loads (pre-transposed on host) ----------------------
        x_dvs = [xd.ap().rearrange("(io p) tb -> p io tb", p=128)
                 for xd in x_ds]
        w1H = wsb.tile([128, KI * H1], F16)
        w1L = wsb.tile([128, KI * H1], F16)
        w1h_v = w1_ds[0].ap().rearrange("(io p) h -> p io h", p=128)
        w1l_v = w1_ds[1].ap().rearrange("(io p) h -> p io h", p=128)
        xT0 = [xt_pool.tile([128, KI * ntb0], F16, tag=f"x{i}",
                            name=f"xT0_{i}")
               for i in range(3)]
        xqs = (nc.sync, nc.gpsimd, nc.scalar)
        for io in range(KI):
            for i in range(3):
                xqs[i].dma_start(out=xT0[i][:, io * ntb0:(io + 1) * ntb0],
                                 in_=x_dvs[i][:, io, 0:ntb0])
            nc.scalar.dma_start(out=w1H[:, io * H1:(io + 1) * H1],
                                in_=w1h_v[:, io, :])
        for io in range(KI):
            nc.scalar.dma_start(out=w1L[:, io * H1:(io + 1) * H1],
                                in_=w1l_v[:, io, :])
        # w2H[p, kj*H2 + h2] = fp16 w2T[kj*128+p, h2]
        # w2LO[p, kj*H2 + h2] = e4m3((w2T - w2h)*2^12), same indexing; the
        # DoubleRow lhsT view pairs kj 2a/2a+1 along a middle dim.
        w2H = wsb.tile([128, J1 * H2], F16)
        w2LO = wsb.tile([128, J1 * H2], F8E4)
        w2h_v = w2h_d.ap().rearrange("(kj p) h -> p kj h", p=128)
        w2lo_v = w2lo_d.ap().rearrange("(kj p) h -> p kj h", p=128)
        for kj in range(J1):
            nc.scalar.dma_start(out=w2H[:, kj * H2:(kj + 1) * H2],
                                in_=w2h_v[:, kj, :])
        for kj in range(J1):
            nc.sync.dma_start(out=w2LO[:, kj * H2:(kj + 1) * H2],
                              in_=w2lo_v[:, kj, :])
        w3hT = wsb.tile([128, J2 * O], F16)
        nc.sync.dma_start(
            out=w3hT.rearrange("p (kj o) -> p kj o", kj=J2),
            in_=w3h_d.ap().rearrange("(kj p) o -> p kj o", p=128))
        b2cT = wsb.tile([128, J2], F32)
        rs3T = wsb.tile([1, O], F16)
        onesT = wsb.tile([1, 128], F16)
        nc.sync.dma_start(out=b2cT, in_=b2c_d.ap())
        nc.sync.dma_start(out=rs3T, in_=rs3h_d.ap())
        nc.sync.dma_start(out=onesT, in_=ones16_d.ap())
        nbias = wsb.tile([128, 1], F32)
        nc.vector.memset(nbias, -1.0)

        # ---- scan scratch -------------------------------------------------
        # LIF state kept NEGATED (nmem = -mem), written in place over cur:
        #   A: tmp   = (nmem * beta) - cur          [= -(beta*mem + cur)]
        #   B: nmem' = (nmem is_lt -1) + tmp        [= -(tmp_ref - spike)]
        # fp32 RNE is sign-symmetric => bit-identical to the reference.
        tmp1 = wsb.tile([128, J1 * BL], F32)
        tmp2 = wsb.tile([128, J2 * BL], F32)
        zf1 = wsb.tile([128, J1 * BL], F32)
        zf2 = wsb.tile([128, J2 * BL], F32)
        nc.vector.memset(zf1, 0.0)
        nc.vector.memset(zf2, 0.0)
        # PE warm-up: trip the HAM clock gate before chunk 0's matmuls.
        nwarm = opts.get("pe_warmup", 16)
        if nwarm:
            wpt = pp3.tile([64, 64], F32, tag="pp3", name="warmpt")
            for _ in range(nwarm):
                nc.tensor.matmul(wpt, lhsT=zf1[:, 0:64], rhs=zf1,
                                 start=True, stop=True)

        toff = [0]
        for s in sched:
            toff.append(toff[-1] + s)

        GG = opts.get("grp", 16)  # spike-extraction group size (steps)

        # ---- per-stage emitters (software-pipelined below) ---------------
        def emit_mm1(c, first=False):
            """fp16 3-pass cur1, one PSUM group per j-tile.
            cur1 scan layout: col = t*64 + j*8 + b   (h1 = j*128 + p)."""
            tcsz = sched[c]
            ntb = tcsz * BL
            tb0 = toff[c] * BL
            if first:
                xs = xT0
            else:
                xs = [xt_pool.tile([128, KI * ntb], F16, tag=f"x{i}",
                                   name=f"xc{c}_{i}")
                      for i in range(3)]
                for i in range(3):
                    xqs[i].dma_start(
                        out=xs[i].rearrange("p (io tb) -> p io tb", io=KI),
                        in_=x_dvs[i][:, :, tb0:tb0 + ntb])
            cur1 = cur1_pool.tile([128, tcsz * J1 * BL], F32, tag="cur1")
            cur1_v = cur1.rearrange("p (t j b) -> p t j b", t=tcsz, j=J1, b=BL)
            for j in range(J1):
                pt = pp1.tile([128, ntb], F32, tag="pp1")
                i_mm = 0
                for io in range(KI):
                    for xi in (0, 1):  # xh, xl vs stationary w1h
                        nc.tensor.matmul(
                            pt,
                            lhsT=w1H[:, io * H1 + j * 128:
                                     io * H1 + (j + 1) * 128],
                            rhs=xs[xi][:, io * ntb:(io + 1) * ntb],
                            start=(i_mm == 0), stop=False)
                        i_mm += 1
                for io in range(KI):  # xh>>10 vs stationary w1l<<10
                    nc.tensor.matmul(
                        pt,
                        lhsT=w1L[:, io * H1 + j * 128:
                                 io * H1 + (j + 1) * 128],
                        rhs=xs[2][:, io * ntb:(io + 1) * ntb],
                        start=False, stop=(io == KI - 1))
                nc.scalar.activation(
                    cur1_v[:, :, j, :],
                    pt.rearrange("p (t b) -> p t b", b=BL), ACTF.Copy)
            return cur1

        def emit_scan1(c, cur1, prev):
            """Serial chain on DVE; spike extraction on Pool (fp16 {0,1});
            s12 = e5m2(spk * 2^-12) derived by ACT copies.  Returns
            (spk1_fp16, s12, cur1-as-nmem)."""
            tcsz = sched[c]
            W = J1 * BL
            spk1 = spk1_pool.tile([128, tcsz * W], F16, tag="spk1")
            s12 = s12_pool.tile([128, tcsz * W], F8E5, tag="s12")
            for t in range(tcsz):
                cs = cur1[:, t * W:(t + 1) * W]
                if t == 0:
                    nprev = zf1 if prev is None else \
                        prev[0][:, (prev[1] - 1) * W: prev[1] * W]
                else:
                    nprev = cur1[:, (t - 1) * W: t * W]
                nc.vector.scalar_tensor_tensor(
                    tmp1, nprev, BETA1, cs, ALU.mult, ALU.subtract)
                nc.vector.scalar_tensor_tensor(
                    cs, nprev, -1.0, tmp1, ALU.is_lt, ALU.add)
                if t % GG == GG - 1 or t == tcsz - 1:
                    g0 = (t // GG) * GG
                    so = spk1[:, g0 * W:(t + 1) * W]
                    si = cur1[:, g0 * W:(t + 1) * W]
                    if ext1 == "act":
                        # s = Sign(-nmem - 1) in {-1, +1} (= 2*spk - 1)
                        nc.scalar.activation(so, si, ACTF.Sign,
                                             bias=nbias, scale=-1.0)
                    elif ext1 == "pool":
                        nc.gpsimd.tensor_scalar(so, si, -1.0, None, ALU.is_lt)
                    else:
                        nc.vector.tensor_scalar(so, si, -1.0, None, ALU.is_lt)
                    nc.scalar.activation(
                        s12[:, g0 * W:(t + 1) * W], so, ACTF.Copy,
                        scale=float(2.0 ** -12))
            return spk1, s12, cur1

        def emit_mm2(c, spk1, s12):
            """fp16-hi (8 matmuls) + e4m3xe5m2 DoubleRow lo (4 matmuls at
            2x rate) per j-tile, one PSUM group."""
            tcsz = sched[c]
            ntb = tcsz * BL
            spk1_v = spk1.rearrange("p (t j b) -> p t j b", t=tcsz, j=J1, b=BL)
            s12_v = s12.rearrange("p (t j b) -> p t j b", t=tcsz, j=J1, b=BL)
            w2lo_v = w2LO.rearrange("p (kj h) -> p kj h", kj=J1)
            cur2 = cur2_pool.tile([128, tcsz * J2 * BL], F32, tag="cur2")
            cur2_v = cur2.rearrange("p (t j b) -> p t j b", t=tcsz, j=J2, b=BL)
            for j in range(J2):
                pt = pp2.tile([128, ntb], F32, tag="pp2")
                for kj in range(J1):
                    nc.tensor.matmul(
                        pt,
                        lhsT=w2H[:, kj * H2 + j * 128: kj * H2 + (j + 1) * 128],
                        rhs=spk1_v[:, :, kj, :],
                        start=(kj == 0), stop=False)
                for a in range(J1 // 2):
                    nc.tensor.matmul(
                        pt,
                        lhsT=w2lo_v[:, 2 * a:2 * a + 2,
                                    j * 128:(j + 1) * 128],
                        rhs=s12_v[:, :, 2 * a:2 * a + 2, :].rearrange(
                            "p t kt b -> p kt t b"),
                        start=False, stop=(a == J1 // 2 - 1),
                        perf_mode=DR, skip_group_check=True)
                if ext1 == "act":
                    # cur2 = 0.5*psum + rowsum(w2_eff)/2  (spikes are +-1)
                    nc.scalar.activation(
                        cur2_v[:, :, j, :],
                        pt.rearrange("p (t b) -> p t b", b=BL), ACTF.Identity,
                        bias=b2cT[:, j:j + 1], scale=0.5)
                else:
                    nc.scalar.activation(
                        cur2_v[:, :, j, :],
                        pt.rearrange("p (t b) -> p t b", b=BL), ACTF.Copy)
            return cur2

        def emit_scan2(c, cur2, prev):
            """Chain on DVE; spk2 extraction on Pool, j-major fp16 out
            (col = j*ntb + t*8 + b) for matmul-3 stationary reads."""
            tcsz = sched[c]
            W = J2 * BL
            ntb = tcsz * BL
            spk2 = spk2_pool.tile([128, tcsz * W], F16, tag="spk2")
            spk2_tv = spk2.rearrange("p (j t b) -> p t j b",
                                     j=J2, t=tcsz, b=BL)
            cur2_v = cur2.rearrange("p (t j b) -> p t j b",
                                    t=tcsz, j=J2, b=BL)
            for t in range(tcsz):
                cs = cur2[:, t * W:(t + 1) * W]
                if t == 0:
                    nprev = zf2 if prev is None else \
                        prev[0][:, (prev[1] - 1) * W: prev[1] * W]
                else:
                    nprev = cur2[:, (t - 1) * W: t * W]
                nc.vector.scalar_tensor_tensor(
                    tmp2, nprev, BETA2, cs, ALU.mult, ALU.subtract)
                nc.vector.scalar_tensor_tensor(
                    cs, nprev, -1.0, tmp2, ALU.is_lt, ALU.add)
                if t % GG == GG - 1 or t == tcsz - 1:
                    g0 = (t // GG) * GG
                    so = spk2_tv[:, g0:t + 1, :, :]
                    si = cur2_v[:, g0:t + 1, :, :]
                    if ext2 == "act":
                        nc.scalar.activation(so, si, ACTF.Sign,
                                             bias=nbias, scale=-1.0)
                    elif ext2 == "pool":
                        nc.gpsimd.tensor_scalar(so, si, -1.0, None, ALU.is_lt)
                    else:
                        nc.vector.tensor_scalar(so, si, -1.0, None, ALU.is_lt)
            return spk2, cur2

        def emit_mm3(c, spk2):
            """fp16 single pass: out[tb, o] = spk2 @ w3^T."""
            tcsz = sched[c]
            ntb = tcsz * BL
            tb0 = toff[c] * BL
            for m0 in range(0, ntb, 128):
                msz = min(128, ntb - m0)
                pt = pp3.tile([msz, O], F32, tag="pp3")
                for kj in range(J2):
                    nc.tensor.matmul(
                        pt,
                        lhsT=spk2[:, kj * ntb + m0: kj * ntb + m0 + msz],
                        rhs=w3hT[:, kj * O:(kj + 1) * O],
                        start=(kj == 0), stop=(kj == J2 - 1 and ext2 != "act"))
                if ext2 == "act":
                    # +rowsum(w3) row: out = (w3 @ s± + rs3) then *0.5
                    nc.tensor.matmul(pt, lhsT=onesT[:, 0:msz], rhs=rs3T,
                                     start=False, stop=True)
                osb = out_pool.tile([msz, O], F32, tag="osb")
                nc.scalar.activation(osb, pt, ACTF.Copy,
                                     scale=(0.5 if ext2 == "act" else 1.0))
                r0 = tb0 + m0
                nc.sync.dma_start(out=y[r0:r0 + msz, :], in_=osb)

        # ---- software-pipelined emission ---------------------------------
        hwloop = opts.get("hwloop", 1)
        loop_cm = tc_ctx.For_i(0, hwloop, 1) if hwloop > 1 else None
        if loop_cm is not None:
            loop_cm.__enter__()
        for _r in range(opts.get("repeat", 1)):
            cur1 = emit_mm1(0, first=(_r == 0 and hwloop == 1))
            spk1, s12, nm1 = emit_scan1(0, cur1, None)
            spk2 = {}
            nm2 = None
            for c in range(nch):
                if c + 1 < nch:
                    cur1 = emit_mm1(c + 1)
                cur2 = emit_mm2(c, spk1, s12)
                if c + 1 < nch:
                    spk1_next, s12_next, nm1_next = emit_scan1(
                        c + 1, cur1, (nm1, sched[c]))
                spk2[c], nm2 = emit_scan2(c, cur2, None if c == 0 else
                                          (nm2, sched[c - 1]))
                if c + 1 < nch:
                    spk1, s12, nm1 = spk1_next, s12_next, nm1_next
            # All matmul-3 tiles run in the epilogue (PE drain filler).
            for cc in range(nch):
                emit_mm3(cc, spk2.pop(cc))
        if loop_cm is not None:
            loop_cm.__exit__(None, None, None)


def prep_inputs(x, w1, w2, w3, n_t=T):
    """Host-side layout prep shared by kernel() and tests."""
    x = np.asarray(x, dtype=np.float32)
    w1 = np.asarray(w1, dtype=np.float32)
    w2 = np.asarray(w2, dtype=np.float32)
    w3 = np.asarray(w3, dtype=np.float32)
    SC = np.float32(2.0 ** 10)
    w1t = np.ascontiguousarray(w1.T)                       # [I, H1] f32
    w1hs = w1t.astype(np.float16)
    w1ls = ((w1t - w1hs.astype(np.float32)) * SC).astype(np.float16)
    w2t = np.ascontiguousarray(w2.T)                       # [H1, H2] f32
    w2hs = w2t.astype(np.float16)
    w2lo = ((w2t - w2hs.astype(np.float32)) * np.float32(2.0 ** 12)) \
        .astype(ml_dtypes.float8_e4m3)
    w3t = np.ascontiguousarray(w3.T)                       # [H2, O] f32
    w3hs = w3t.astype(np.float16)
    w2_eff = w2hs.astype(np.float64) + w2lo.astype(np.float64) * 2.0 ** -12
    rs2 = w2_eff.sum(axis=0)                               # [H2]
    b2c = (rs2 * 0.5).astype(np.float32).reshape(J2, 128).T  # [128, J2]
    rs3 = w3hs.astype(np.float64).sum(axis=0)              # [O]
    common = {
        "w1h": w1hs,
        "w1l10": w1ls,
        "w2h": w2hs,
        "w2lo": w2lo,
        "w3h": w3hs,
        "b2c": np.ascontiguousarray(b2c),
        "rs3h": rs3.astype(np.float16).reshape(1, O),
        "ones16": np.ones((1, 128), np.float16),
    }
    xcores = []
    for cid in range(NCORES):
        xs = x[:, cid * BL:(cid + 1) * BL, :].reshape(n_t * BL, I)
        xT = np.ascontiguousarray(xs.T)                    # [I, n_t*BL] f32
        xh = xT.astype(np.float16)
        xl = (xT - xh.astype(np.float32)).astype(np.float16)
        xh10 = (xh.astype(np.float32) / SC).astype(np.float16)
        xcores.append({"xh": xh, "xl": xl, "xh10": xh10})
    return xcores, common


_NC_CACHE = {}


def _get_nc():
    if "nc" not in _NC_CACHE:
        _NC_CACHE["nc"] = build()
    return _NC_CACHE["nc"]


def kernel(x, w1, b1, w2, b2, w3, b3, **_unused):
    """Full inputs in, full output out. b1/b2/b3 are zeros in this problem
    (asserted) -- the device program skips the bias adds."""
    assert not np.any(np.asarray(b1)) and not np.any(np.asarray(b2)) \
        and not np.any(np.asarray(b3)), "nonzero biases unsupported"

    nc = _get_nc()
    xcores, common = prep_inputs(x, w1, w2, w3)
    in_maps = [{**xcores[cid], **common} for cid in range(NCORES)]
    res = run_bass_kernel_spmd(nc, in_maps, list(range(NCORES)))
    outs = [r["y"].reshape(T, BL, O) for r in res.results]
    return np.concatenate(outs, axis=1)


if __name__ == "__main__":
    nc = build()
    print("built OK")


# revision 6
# speedup vs baseline: 1.0599x; 1.0599x over previous
"""Trainium2 Bass kernel for a 3-layer spiking net (snntorch-style Leaky/LIF).

Math (per timestep t, eval mode):
    cur1 = x_t @ w1.T + b1
    mem1 = 0.9*mem1 + cur1 - (mem1_prev > 1)        # reset-by-subtract
    spk1 = (mem1 > 1)
    cur2 = spk1 @ w2.T + b2
    mem2 = 0.85*mem2 + cur2 - (mem2_prev > 1)
    spk2 = (mem2 > 1)
    out_t = spk2 @ w3.T + b3

Strategy (v2):
  - Data-parallel over batch: B=64 -> 8 cores x 8.
  - Matmuls batched over T in chunks; only the elementwise LIF updates are
    sequential.  The serial chains (2 scalar_tensor_tensor ops per step per
    layer, fp32, bit-exact vs the reference via the negated-membrane trick)
    run on the DVE; NOTHING else runs on the DVE, so the chain streams
    back-to-back (~127ns/op, 1024 ops ~ 130us).
  - Spike extraction (batched is_lt every 16 steps) runs on the Pool
    engine (gpsimd), which is otherwise idle -- it no longer interrupts
    the DVE chain.  spk tiles are fp16 {0,1} (exact).
  - Matmul 1: fp16 3-pass as before (x hi/lo + w1 lo with 2^10 shifts),
    known band ~2.7e-7.  82us of PE.
  - Matmul 2: fp16-hi + fp8-lo: cur2 = w2h^T s + (w2lo8^T s12) where
    w2h = fp16(w2), w2lo8 = e4m3((w2 - w2h) * 2^12) via a DoubleRow
    (2x-rate) fp8 matmul against s12 = e5m2(spk * 2^-12) (exact: powers
    of two).  Effective w2 residual ~2^-21, same as the old bf16 hi/lo
    2-pass, at 10/16 the PE cycles (68us).  Probe-verified exact on HW.
  - Matmul 3: fp16 w3 single pass (output error ~1e-4, was 1.5e-3 bf16).
  - s12 tiles are derived from the fp16 spikes by ACT copies with scale
    2^-12 (exact, probe-verified).
  - Pipeline: MM1(c+1) -> MM2(c) on PE; scan1(c+1) before scan2(c) on
    DVE; MM3 in the epilogue to fill the PE drain window.
"""

import sys

for _p in ("/opt/trn_rl_repo", "/root/.axon_site/_ro/pypackages"):
    if _p not in sys.path:
        sys.path.insert(0, _p)

import ml_dtypes
import numpy as np

import concourse.bass as bass
import concourse.mybir as mybir
from concourse import bacc, tile
from concourse.bass_utils import run_bass_kernel_spmd

F32 = mybir.dt.float32
F16 = mybir.dt.float16
BF16 = mybir.dt.bfloat16
F8E4 = mybir.dt.float8e4
F8E5 = mybir.dt.float8e5
ALU = mybir.AluOpType
ACTF = mybir.ActivationFunctionType
DR = mybir.MatmulPerfMode.DoubleRow

# Problem shape (hardcoded; harness runs kernel.py standalone).
T, B, I, H1, H2, O = 256, 64, 512, 1024, 1024, 256
NCORES = 8
BL = B // NCORES          # batch per core
BETA1, BETA2 = 0.9, 0.85
KI = I // 128             # K-tiles for matmul 1 (4)
J1 = H1 // 128            # M-tiles for layer 1 (8)
J2 = H2 // 128            # M-tiles for layer 2 (8)


def build(n_t=T, sched=None, trace_sim=False, opts=None):
    """Build the per-core SPMD program. Identical on all cores."""
    if sched is None:
        sched = (opts or {}).get("sched") or [32, 64, 64, 64, 32]
    assert sum(sched) == n_t and all(s % 8 == 0 for s in sched)
    nc = bacc.Bacc("TRN2", target_bir_lowering=False, debug=False)

    xh = nc.declare_dram_parameter("xh", [I, n_t * BL], F16, isOutput=False)
    xl = nc.declare_dram_parameter("xl", [I, n_t * BL], F16, isOutput=False)
    xh10 = nc.declare_dram_parameter("xh10", [I, n_t * BL], F16,
                                     isOutput=False)
    w1h = nc.declare_dram_parameter("w1h", [I, H1], F16, isOutput=False)
    w1l10 = nc.declare_dram_parameter("w1l10", [I, H1], F16, isOutput=False)
    w2h = nc.declare_dram_parameter("w2h", [H1, H2], F16, isOutput=False)
    w2lo = nc.declare_dram_parameter("w2lo", [H1, H2], F8E4, isOutput=False)
    w3h = nc.declare_dram_parameter("w3h", [H2, O], F16, isOutput=False)
    b2c = nc.declare_dram_parameter("b2c", [128, J2], F32, isOutput=False)
    rs3h = nc.declare_dram_parameter("rs3h", [1, O], F16, isOutput=False)
    ones16 = nc.declare_dram_parameter("ones16", [1, 128], F16,
                                       isOutput=False)
    y = nc.declare_dram_parameter("y", [n_t * BL, O], F32, isOutput=True)

    with tile.TileContext(nc, trace_sim=trace_sim) as tc_ctx:
        _body(nc, tc_ctx, (xh, xl, xh10), (w1h, w1l10), w2h, w2lo, w3h,
              (b2c, rs3h, ones16), y, n_t, sched, opts or {})
    nc.compile()
    return nc


def _body(nc, tc_ctx, x_ds, w1_ds, w2h_d, w2lo_d, w3h_d, corr_ds, y,
          n_t, sched, opts):
    b2c_d, rs3h_d, ones16_d = corr_ds
    ext = opts.get("ext", "dve")
    ext1 = opts.get("ext1", ext)
    ext2 = opts.get("ext2", ext)
    nch = len(sched)
    ntb0 = sched[0] * BL
    import contextlib

    ctx = contextlib.ExitStack()
    with ctx:
        cb = opts.get("cur_bufs", 2)
        pb1, pb2, pb3 = opts.get("psum_bufs", (3, 3, 2))
        wsb = ctx.enter_context(tc_ctx.tile_pool(name="wsb", bufs=1))
        xt_pool = ctx.enter_context(
            tc_ctx.tile_pool(name="xt", bufs=opts.get("xt_bufs", 2)))
        cur1_pool = ctx.enter_context(tc_ctx.tile_pool(name="cur1", bufs=cb))
        spk1_pool = ctx.enter_context(tc_ctx.tile_pool(name="spk1", bufs=2))
        s12_pool = ctx.enter_context(tc_ctx.tile_pool(name="s12", bufs=2))
        cur2_pool = ctx.enter_context(tc_ctx.tile_pool(name="cur2", bufs=cb))
        spk2_pool = ctx.enter_context(tc_ctx.tile_pool(name="spk2",
                                                       bufs=len(sched)))
        out_pool = ctx.enter_context(tc_ctx.tile_pool(name="outp", bufs=3))
        pp1 = ctx.enter_context(tc_ctx.tile_pool(name="pp1", bufs=pb1,
                                                 space="PSUM"))
        pp2 = ctx.enter_context(tc_ctx.tile_pool(name="pp2", bufs=pb2,
                                                 space="PSUM"))
        pp3 = ctx.enter_context(tc_ctx.tile_pool(name="pp3", bufs=pb3,
                                                 space="PSUM"))

        # ---- weight loads (pre-transposed on host) ----------------------
        x_dvs = [xd.ap().rearrange("(io p) tb -> p io tb", p=128)
                 for xd in x_ds]
        w1H = wsb.tile([128, KI * H1], F16)
        w1L = wsb.tile([128, KI * H1], F16)
        w1h_v = w1_ds[0].ap().rearrange("(io p) h -> p io h", p=128)
        w1l_v = w1_ds[1].ap().rearrange("(io p) h -> p io h", p=128)
        xT0 = [xt_pool.tile([128, KI * ntb0], F16, tag=f"x{i}",
                            name=f"xT0_{i}")
               for i in range(3)]
        xqs = (nc.sync, nc.gpsimd, nc.scalar)
        for io in range(KI):
            for i in range(3):
                xqs[i].dma_start(out=xT0[i][:, io * ntb0:(io + 1) * ntb0],
                                 in_=x_dvs[i][:, io, 0:ntb0])
            nc.scalar.dma_start(out=w1H[:, io * H1:(io + 1) * H1],
                                in_=w1h_v[:, io, :])
        for io in range(KI):
            nc.scalar.dma_start(out=w1L[:, io * H1:(io + 1) * H1],
                                in_=w1l_v[:, io, :])
        # w2H[p, kj*H2 + h2] = fp16 w2T[kj*128+p, h2]
        # w2LO[p, kj*H2 + h2] = e4m3((w2T - w2h)*2^12), same indexing; the
        # DoubleRow lhsT view pairs kj 2a/2a+1 along a middle dim.
        w2H = wsb.tile([128, J1 * H2], F16)
        w2LO = wsb.tile([128, J1 * H2], F8E4)
        w2h_v = w2h_d.ap().rearrange("(kj p) h -> p kj h", p=128)
        w2lo_v = w2lo_d.ap().rearrange("(kj p) h -> p kj h", p=128)
        for kj in range(J1):
            nc.scalar.dma_start(out=w2H[:, kj * H2:(kj + 1) * H2],
                                in_=w2h_v[:, kj, :])
        for kj in range(J1):
            nc.sync.dma_start(out=w2LO[:, kj * H2:(kj + 1) * H2],
                              in_=w2lo_v[:, kj, :])
        w3hT = wsb.tile([128, J2 * O], F16)
        nc.sync.dma_start(
            out=w3hT.rearrange("p (kj o) -> p kj o", kj=J2),
            in_=w3h_d.ap().rearrange("(kj p) o -> p kj o", p=128))
        b2cT = wsb.tile([128, J2], F32)
        rs3T = wsb.tile([1, O], F16)
        onesT = wsb.tile([1, 128], F16)
        nc.sync.dma_start(out=b2cT, in_=b2c_d.ap())
        nc.sync.dma_start(out=rs3T, in_=rs3h_d.ap())
        nc.sync.dma_start(out=onesT, in_=ones16_d.ap())
        nbias = wsb.tile([128, 1], F32)
        nc.vector.memset(nbias, -1.0)

        # ---- scan scratch -------------------------------------------------
        # LIF state kept NEGATED (nmem = -mem), written in place over cur:
        #   A: tmp   = (nmem * beta) - cur          [= -(beta*mem + cur)]
        #   B: nmem' = (nmem is_lt -1) + tmp        [= -(tmp_ref - spike)]
        # fp32 RNE is sign-symmetric => bit-identical to the reference.
        tmp1 = wsb.tile([128, J1 * BL], F32)
        tmp2 = wsb.tile([128, J2 * BL], F32)
        zf1 = wsb.tile([128, J1 * BL], F32)
        zf2 = wsb.tile([128, J2 * BL], F32)
        nc.vector.memset(zf1, 0.0)
        nc.vector.memset(zf2, 0.0)
        # PE warm-up: trip the HAM clock gate before chunk 0's matmuls.
        nwarm = opts.get("pe_warmup", 16)
        if nwarm:
            wpt = pp3.tile([64, 64], F32, tag="pp3", name="warmpt")
            for _ in range(nwarm):
                nc.tensor.matmul(wpt, lhsT=zf1[:, 0:64], rhs=zf1,
                                 start=True, stop=True)

        toff = [0]
        for s in sched:
            toff.append(toff[-1] + s)

        GG = opts.get("grp", 16)  # spike-extraction group size (steps)

        # ---- per-stage emitters (software-pipelined below) ---------------
        def emit_mm1(c, first=False):
            """fp16 3-pass cur1, one PSUM group per j-tile.
            cur1 scan layout: col = t*64 + j*8 + b   (h1 = j*128 + p)."""
            tcsz = sched[c]
            ntb = tcsz * BL
            tb0 = toff[c] * BL
            if first:
                xs = xT0
            else:
                xs = [xt_pool.tile([128, KI * ntb], F16, tag=f"x{i}",
                                   name=f"xc{c}_{i}")
                      for i in range(3)]
                for i in range(3):
                    xqs[i].dma_start(
                        out=xs[i].rearrange("p (io tb) -> p io tb", io=KI),
                        in_=x_dvs[i][:, :, tb0:tb0 + ntb])
            cur1 = cur1_pool.tile([128, tcsz * J1 * BL], F32, tag="cur1")
            cur1_v = cur1.rearrange("p (t j b) -> p t j b", t=tcsz, j=J1, b=BL)
            for j in range(J1):
                pt = pp1.tile([128, ntb], F32, tag="pp1")
                i_mm = 0
                for io in range(KI):
                    for xi in (0, 1):  # xh, xl vs stationary w1h
                        nc.tensor.matmul(
                            pt,
                            lhsT=w1H[:, io * H1 + j * 128:
                                     io * H1 + (j + 1) * 128],
                            rhs=xs[xi][:, io * ntb:(io + 1) * ntb],
                            start=(i_mm == 0), stop=False)
                        i_mm += 1
                for io in range(KI):  # xh>>10 vs stationary w1l<<10
                    nc.tensor.matmul(
                        pt,
                        lhsT=w1L[:, io * H1 + j * 128:
                                 io * H1 + (j + 1) * 128],
                        rhs=xs[2][:, io * ntb:(io + 1) * ntb],
                        start=False, stop=(io == KI - 1))
                nc.scalar.activation(
                    cur1_v[:, :, j, :],
                    pt.rearrange("p (t b) -> p t b", b=BL), ACTF.Copy)
            return cur1

        def scan1_state(c, cur1):
            """Allocate chunk-c scan-1 outputs."""
            tcsz = sched[c]
            W = J1 * BL
            spk1 = spk1_pool.tile([128, tcsz * W], F16, tag="spk1")
            s12 = s12_pool.tile([128, tcsz * W], F8E5, tag="s12")
            return dict(c=c, t=0, tcsz=tcsz, W=W, cur=cur1, spk=spk1,
                        s12=s12)

        def scan1_step(st, prev):
            """One LIF step of layer 1 (A+B on DVE); extraction at group
            boundaries.  prev = (nmem tile of chunk c-1, its tcsz)."""
            t, W, cur1 = st["t"], st["W"], st["cur"]
            cs = cur1[:, t * W:(t + 1) * W]
            if t == 0:
                nprev = zf1 if prev is None else \
                    prev[0][:, (prev[1] - 1) * W: prev[1] * W]
            else:
                nprev = cur1[:, (t - 1) * W: t * W]
            nc.vector.scalar_tensor_tensor(
                tmp1, nprev, BETA1, cs, ALU.mult, ALU.subtract)
            nc.vector.scalar_tensor_tensor(
                cs, nprev, -1.0, tmp1, ALU.is_lt, ALU.add)
            if t % GG == GG - 1 or t == st["tcsz"] - 1:
                g0 = (t // GG) * GG
                so = st["spk"][:, g0 * W:(t + 1) * W]
                si = cur1[:, g0 * W:(t + 1) * W]
                if ext1 == "act":
                    nc.scalar.activation(so, si, ACTF.Sign,
                                         bias=nbias, scale=-1.0)
                elif ext1 == "pool":
                    nc.gpsimd.tensor_scalar(so, si, -1.0, None, ALU.is_lt)
                else:
                    nc.vector.tensor_scalar(so, si, -1.0, None, ALU.is_lt)
                nc.scalar.activation(
                    st["s12"][:, g0 * W:(t + 1) * W], so, ACTF.Copy,
                    scale=float(2.0 ** -12))
            st["t"] = t + 1

        def emit_mm2(c, spk1, s12):
            """fp16-hi (8 matmuls) + e4m3xe5m2 DoubleRow lo (4 matmuls at
            2x rate) per j-tile, one PSUM group."""
            tcsz = sched[c]
            ntb = tcsz * BL
            spk1_v = spk1.rearrange("p (t j b) -> p t j b", t=tcsz, j=J1, b=BL)
            s12_v = s12.rearrange("p (t j b) -> p t j b", t=tcsz, j=J1, b=BL)
            w2lo_v = w2LO.rearrange("p (kj h) -> p kj h", kj=J1)
            cur2 = cur2_pool.tile([128, tcsz * J2 * BL], F32, tag="cur2")
            cur2_v = cur2.rearrange("p (t j b) -> p t j b", t=tcsz, j=J2, b=BL)
            for j in range(J2):
                pt = pp2.tile([128, ntb], F32, tag="pp2")
                for kj in range(J1):
                    nc.tensor.matmul(
                        pt,
                        lhsT=w2H[:, kj * H2 + j * 128: kj * H2 + (j + 1) * 128],
                        rhs=spk1_v[:, :, kj, :],
                        start=(kj == 0), stop=False)
                for a in range(J1 // 2):
                    nc.tensor.matmul(
                        pt,
                        lhsT=w2lo_v[:, 2 * a:2 * a + 2,
                                    j * 128:(j + 1) * 128],
                        rhs=s12_v[:, :, 2 * a:2 * a + 2, :].rearrange(
                            "p t kt b -> p kt t b"),
                        start=False, stop=(a == J1 // 2 - 1),
                        perf_mode=DR, skip_group_check=True)
                if ext1 == "act":
                    # cur2 = 0.5*psum + rowsum(w2_eff)/2  (spikes are +-1)
                    nc.scalar.activation(
                        cur2_v[:, :, j, :],
                        pt.rearrange("p (t b) -> p t b", b=BL), ACTF.Identity,
                        bias=b2cT[:, j:j + 1], scale=0.5)
                else:
                    nc.scalar.activation(
                        cur2_v[:, :, j, :],
                        pt.rearrange("p (t b) -> p t b", b=BL), ACTF.Copy)
            return cur2

        def scan2_state(c, cur2):
            tcsz = sched[c]
            W = J2 * BL
            spk2 = spk2_pool.tile([128, tcsz * W], F16, tag="spk2")
            return dict(c=c, t=0, tcsz=tcsz, W=W, cur=cur2, spk=spk2)

        def scan2_step(st, prev):
            """One LIF step of layer 2; spk2 written j-major
            (col = j*ntb + t*8 + b) for matmul-3 stationary reads."""
            t, W, cur2 = st["t"], st["W"], st["cur"]
            tcsz = st["tcsz"]
            cs = cur2[:, t * W:(t + 1) * W]
            if t == 0:
                nprev = zf2 if prev is None else \
                    prev[0][:, (prev[1] - 1) * W: prev[1] * W]
            else:
                nprev = cur2[:, (t - 1) * W: t * W]
            nc.vector.scalar_tensor_tensor(
                tmp2, nprev, BETA2, cs, ALU.mult, ALU.subtract)
            nc.vector.scalar_tensor_tensor(
                cs, nprev, -1.0, tmp2, ALU.is_lt, ALU.add)
            if t % GG == GG - 1 or t == tcsz - 1:
                g0 = (t // GG) * GG
                spk2_tv = st["spk"].rearrange("p (j t b) -> p t j b",
                                              j=J2, t=tcsz, b=BL)
                cur2_v = cur2.rearrange("p (t j b) -> p t j b",
                                        t=tcsz, j=J2, b=BL)
                so = spk2_tv[:, g0:t + 1, :, :]
                si = cur2_v[:, g0:t + 1, :, :]
                if ext2 == "act":
                    nc.scalar.activation(so, si, ACTF.Sign,
                                         bias=nbias, scale=-1.0)
                elif ext2 == "pool":
                    nc.gpsimd.tensor_scalar(so, si, -1.0, None, ALU.is_lt)
                else:
                    nc.vector.tensor_scalar(so, si, -1.0, None, ALU.is_lt)
            st["t"] = t + 1

        def emit_mm3(c, spk2):
            """fp16 single pass: out[tb, o] = spk2 @ w3^T."""
            tcsz = sched[c]
            ntb = tcsz * BL
            tb0 = toff[c] * BL
            for m0 in range(0, ntb, 128):
                msz = min(128, ntb - m0)
                pt = pp3.tile([msz, O], F32, tag="pp3")
                for kj in range(J2):
                    nc.tensor.matmul(
                        pt,
                        lhsT=spk2[:, kj * ntb + m0: kj * ntb + m0 + msz],
                        rhs=w3hT[:, kj * O:(kj + 1) * O],
                        start=(kj == 0), stop=(kj == J2 - 1 and ext2 != "act"))
                if ext2 == "act":
                    # +rowsum(w3) row: out = (w3 @ s± + rs3) then *0.5
                    nc.tensor.matmul(pt, lhsT=onesT[:, 0:msz], rhs=rs3T,
                                     start=False, stop=True)
                osb = out_pool.tile([msz, O], F32, tag="osb")
                nc.scalar.activation(osb, pt, ACTF.Copy,
                                     scale=(0.5 if ext2 == "act" else 1.0))
                r0 = tb0 + m0
                nc.sync.dma_start(out=y[r0:r0 + msz, :], in_=osb)

        # ---- software-pipelined emission ---------------------------------
        hwloop = opts.get("hwloop", 1)
        loop_cm = tc_ctx.For_i(0, hwloop, 1) if hwloop > 1 else None
        if loop_cm is not None:
            loop_cm.__enter__()
        for _r in range(opts.get("repeat", 1)):
            cur1 = emit_mm1(0, first=(_r == 0 and hwloop == 1))
            st1 = scan1_state(0, cur1)
            nm1 = prev1 = None
            while st1["t"] < st1["tcsz"]:
                scan1_step(st1, None)
            nm1 = (st1["cur"], st1["tcsz"])
            spk2 = {}
            nm2 = None
            st2 = None
            for c in range(nch):
                if c + 1 < nch:
                    cur1 = emit_mm1(c + 1)
                cur2 = emit_mm2(c, st1["spk"], st1["s12"])
                st2 = scan2_state(c, cur2)
                if c + 1 < nch:
                    st1n = scan1_state(c + 1, cur1)
                    # op-by-op interleave of the two independent chains:
                    # the DVE pipelines alternating-stream instructions
                    # (~144ns/op vs ~341ns serial, HW-measured).
                    while st1n["t"] < st1n["tcsz"] or st2["t"] < st2["tcsz"]:
                        if st1n["t"] < st1n["tcsz"]:
                            scan1_step(st1n, nm1)
                        if st2["t"] < st2["tcsz"]:
                            scan2_step(st2, nm2)
                    nm1 = (st1n["cur"], st1n["tcsz"])
                    st1 = st1n
                else:
                    while st2["t"] < st2["tcsz"]:
                        scan2_step(st2, nm2)
                spk2[c] = st2["spk"]
                nm2 = (st2["cur"], st2["tcsz"])
            # All matmul-3 tiles run in the epilogue (PE drain filler).
            for cc in range(nch):
                emit_mm3(cc, spk2.pop(cc))
        if loop_cm is not None:
            loop_cm.__exit__(None, None, None)


def prep_inputs(x, w1, w2, w3, n_t=T):
    """Host-side layout prep shared by kernel() and tests."""
    x = np.asarray(x, dtype=np.float32)
    w1 = np.asarray(w1, dtype=np.float32)
    w2 = np.asarray(w2, dtype=np.float32)
    w3 = np.asarray(w3, dtype=np.float32)
    SC = np.float32(2.0 ** 10)
    w1t = np.ascontiguousarray(w1.T)                       # [I, H1] f32
    w1hs = w1t.astype(np.float16)
    w1ls = ((w1t - w1hs.astype(np.float32)) * SC).astype(np.float16)
    w2t = np.ascontiguousarray(w2.T)                       # [H1, H2] f32
    w2hs = w2t.astype(np.float16)
    w2lo = ((w2t - w2hs.astype(np.float32)) * np.float32(2.0 ** 12)) \
        .astype(ml_dtypes.float8_e4m3)
    w3t = np.ascontiguousarray(w3.T)                       # [H2, O] f32
    w3hs = w3t.astype(np.float16)
    w2_eff = w2hs.astype(np.float64) + w2lo.astype(np.float64) * 2.0 ** -12
    rs2 = w2_eff.sum(axis=0)                               # [H2]
    b2c = (rs2 * 0.5).astype(np.float32).reshape(J2, 128).T  # [128, J2]
    rs3 = w3hs.astype(np.float64).sum(axis=0)              # [O]
    common = {
        "w1h": w1hs,
        "w1l10": w1ls,
        "w2h": w2hs,
        "w2lo": w2lo,
        "w3h": w3hs,
        "b2c": np.ascontiguousarray(b2c),
        "rs3h": rs3.astype(np.float16).reshape(1, O),
        "ones16": np.ones((1, 128), np.float16),
    }
    xcores = []
    for cid in range(NCORES):
        xs = x[:, cid * BL:(cid + 1) * BL, :].reshape(n_t * BL, I)
        xT = np.ascontiguousarray(xs.T)                    # [I, n_t*BL] f32
        xh = xT.astype(np.float16)
        xl = (xT - xh.astype(np.float32)).astype(np.float16)
        xh10 = (xh.astype(np.float32) / SC).astype(np.float16)
        xcores.append({"xh": xh, "xl": xl, "xh10": xh10})
    return xcores, common


_NC_CACHE = {}


def _get_nc():
    if "nc" not in _NC_CACHE:
        _NC_CACHE["nc"] = build()
    return _NC_CACHE["nc"]


def kernel(x, w1, b1, w2, b2, w3, b3, **_unused):
    """Full inputs in, full output out. b1/b2/b3 are zeros in this problem
    (asserted) -- the device program skips the bias adds."""
    assert not np.any(np.asarray(b1)) and not np.any(np.asarray(b2)) \
        and not np.any(np.asarray(b3)), "nonzero biases unsupported"

    nc = _get_nc()
    xcores, common = prep_inputs(x, w1, w2, w3)
    in_maps = [{**xcores[cid], **common} for cid in range(NCORES)]
    res = run_bass_kernel_spmd(nc, in_maps, list(range(NCORES)))
    outs = [r["y"].reshape(T, BL, O) for r in res.results]
    return np.concatenate(outs, axis=1)


if __name__ == "__main__":
    nc = build()
    print("built OK")


# revision 8
# speedup vs baseline: 1.0640x; 1.0038x over previous
"""Trainium2 Bass kernel for a 3-layer spiking net (snntorch-style Leaky/LIF).

Math (per timestep t, eval mode):
    cur1 = x_t @ w1.T + b1
    mem1 = 0.9*mem1 + cur1 - (mem1_prev > 1)        # reset-by-subtract
    spk1 = (mem1 > 1)
    cur2 = spk1 @ w2.T + b2
    mem2 = 0.85*mem2 + cur2 - (mem2_prev > 1)
    spk2 = (mem2 > 1)
    out_t = spk2 @ w3.T + b3

Strategy (v2):
  - Data-parallel over batch: B=64 -> 8 cores x 8.
  - Matmuls batched over T in chunks; only the elementwise LIF updates are
    sequential.  The serial chains (2 scalar_tensor_tensor ops per step per
    layer, fp32, bit-exact vs the reference via the negated-membrane trick)
    run on the DVE; NOTHING else runs on the DVE, so the chain streams
    back-to-back (~127ns/op, 1024 ops ~ 130us).
  - Spike extraction (batched is_lt every 16 steps) runs on the Pool
    engine (gpsimd), which is otherwise idle -- it no longer interrupts
    the DVE chain.  spk tiles are fp16 {0,1} (exact).
  - Matmul 1: fp16 3-pass as before (x hi/lo + w1 lo with 2^10 shifts),
    known band ~2.7e-7.  82us of PE.
  - Matmul 2: fp16-hi + fp8-lo: cur2 = w2h^T s + (w2lo8^T s12) where
    w2h = fp16(w2), w2lo8 = e4m3((w2 - w2h) * 2^12) via a DoubleRow
    (2x-rate) fp8 matmul against s12 = e5m2(spk * 2^-12) (exact: powers
    of two).  Effective w2 residual ~2^-21, same as the old bf16 hi/lo
    2-pass, at 10/16 the PE cycles (68us).  Probe-verified exact on HW.
  - Matmul 3: fp16 w3 single pass (output error ~1e-4, was 1.5e-3 bf16).
  - s12 tiles are derived from the fp16 spikes by ACT copies with scale
    2^-12 (exact, probe-verified).
  - Pipeline: MM1(c+1) -> MM2(c) on PE; scan1(c+1) before scan2(c) on
    DVE; MM3 in the epilogue to fill the PE drain window.
"""

import sys

for _p in ("/opt/trn_rl_repo", "/root/.axon_site/_ro/pypackages"):
    if _p not in sys.path:
        sys.path.insert(0, _p)

import ml_dtypes
import numpy as np

import concourse.bass as bass
import concourse.mybir as mybir
from concourse import bacc, tile
from concourse.bass_utils import run_bass_kernel_spmd

F32 = mybir.dt.float32
F16 = mybir.dt.float16
BF16 = mybir.dt.bfloat16
F8E4 = mybir.dt.float8e4
F8E5 = mybir.dt.float8e5
ALU = mybir.AluOpType
ACTF = mybir.ActivationFunctionType
DR = mybir.MatmulPerfMode.DoubleRow

# Problem shape (hardcoded; harness runs kernel.py standalone).
T, B, I, H1, H2, O = 256, 64, 512, 1024, 1024, 256
NCORES = 8
BL = B // NCORES          # batch per core
BETA1, BETA2 = 0.9, 0.85
KI = I // 128             # K-tiles for matmul 1 (4)
J1 = H1 // 128            # M-tiles for layer 1 (8)
J2 = H2 // 128            # M-tiles for layer 2 (8)


def build(n_t=T, sched=None, trace_sim=False, opts=None):
    """Build the per-core SPMD program. Identical on all cores."""
    if sched is None:
        sched = (opts or {}).get("sched") or [32, 64, 64, 64, 32]
    assert sum(sched) == n_t and all(s % 8 == 0 for s in sched)
    nc = bacc.Bacc("TRN2", target_bir_lowering=False, debug=False)

    xh = nc.declare_dram_parameter("xh", [I, n_t * BL], F16, isOutput=False)
    xl = nc.declare_dram_parameter("xl", [I, n_t * BL], F16, isOutput=False)
    xh10 = nc.declare_dram_parameter("xh10", [I, n_t * BL], F16,
                                     isOutput=False)
    w1h = nc.declare_dram_parameter("w1h", [I, H1], F16, isOutput=False)
    w1l10 = nc.declare_dram_parameter("w1l10", [I, H1], F16, isOutput=False)
    w2h = nc.declare_dram_parameter("w2h", [H1, H2], F16, isOutput=False)
    w2lo = nc.declare_dram_parameter("w2lo", [H1, H2], F8E4, isOutput=False)
    w3h = nc.declare_dram_parameter("w3h", [H2, O], F16, isOutput=False)
    b2c = nc.declare_dram_parameter("b2c", [128, J2], F32, isOutput=False)
    rs3h = nc.declare_dram_parameter("rs3h", [1, O], F16, isOutput=False)
    ones16 = nc.declare_dram_parameter("ones16", [1, 128], F16,
                                       isOutput=False)
    y = nc.declare_dram_parameter("y", [n_t * BL, O], F32, isOutput=True)

    with tile.TileContext(nc, trace_sim=trace_sim) as tc_ctx:
        _body(nc, tc_ctx, (xh, xl, xh10), (w1h, w1l10), w2h, w2lo, w3h,
              (b2c, rs3h, ones16), y, n_t, sched, opts or {})
    nc.compile()
    return nc


def _body(nc, tc_ctx, x_ds, w1_ds, w2h_d, w2lo_d, w3h_d, corr_ds, y,
          n_t, sched, opts):
    b2c_d, rs3h_d, ones16_d = corr_ds
    ext = opts.get("ext", "dve")
    ext1 = opts.get("ext1", ext)
    ext2 = opts.get("ext2", ext)
    nch = len(sched)
    ntb0 = sched[0] * BL
    import contextlib

    ctx = contextlib.ExitStack()
    with ctx:
        cb = opts.get("cur_bufs", 2)
        pb1, pb2, pb3 = opts.get("psum_bufs", (3, 3, 2))
        wsb = ctx.enter_context(tc_ctx.tile_pool(name="wsb", bufs=1))
        xt_pool = ctx.enter_context(
            tc_ctx.tile_pool(name="xt", bufs=opts.get("xt_bufs", 2)))
        cur1_pool = ctx.enter_context(tc_ctx.tile_pool(name="cur1", bufs=cb))
        spk1_pool = ctx.enter_context(tc_ctx.tile_pool(name="spk1", bufs=2))
        s12_pool = ctx.enter_context(tc_ctx.tile_pool(name="s12", bufs=2))
        cur2_pool = ctx.enter_context(tc_ctx.tile_pool(name="cur2", bufs=cb))
        spk2_pool = ctx.enter_context(tc_ctx.tile_pool(name="spk2",
                                                       bufs=len(sched)))
        out_pool = ctx.enter_context(tc_ctx.tile_pool(name="outp", bufs=3))
        pp1 = ctx.enter_context(tc_ctx.tile_pool(name="pp1", bufs=pb1,
                                                 space="PSUM"))
        pp2 = ctx.enter_context(tc_ctx.tile_pool(name="pp2", bufs=pb2,
                                                 space="PSUM"))
        pp3 = ctx.enter_context(tc_ctx.tile_pool(name="pp3", bufs=pb3,
                                                 space="PSUM"))

        # ---- weight loads (pre-transposed on host) ----------------------
        x_dvs = [xd.ap().rearrange("(io p) tb -> p io tb", p=128)
                 for xd in x_ds]
        w1H = wsb.tile([128, KI * H1], F16)
        w1L = wsb.tile([128, KI * H1], F16)
        w1h_v = w1_ds[0].ap().rearrange("(io p) h -> p io h", p=128)
        w1l_v = w1_ds[1].ap().rearrange("(io p) h -> p io h", p=128)
        xT0 = [xt_pool.tile([128, KI * ntb0], F16, tag=f"x{i}",
                            name=f"xT0_{i}")
               for i in range(3)]
        xqs = (nc.sync, nc.gpsimd, nc.scalar)
        for io in range(KI):
            for i in range(3):
                xqs[i].dma_start(out=xT0[i][:, io * ntb0:(io + 1) * ntb0],
                                 in_=x_dvs[i][:, io, 0:ntb0])
            nc.scalar.dma_start(out=w1H[:, io * H1:(io + 1) * H1],
                                in_=w1h_v[:, io, :])
        for io in range(KI):
            nc.scalar.dma_start(out=w1L[:, io * H1:(io + 1) * H1],
                                in_=w1l_v[:, io, :])
        # w2H[p, kj*H2 + h2] = fp16 w2T[kj*128+p, h2]
        # w2LO[p, kj*H2 + h2] = e4m3((w2T - w2h)*2^12), same indexing; the
        # DoubleRow lhsT view pairs kj 2a/2a+1 along a middle dim.
        w2H = wsb.tile([128, J1 * H2], F16)
        w2LO = wsb.tile([128, J1 * H2], F8E4)
        w2h_v = w2h_d.ap().rearrange("(kj p) h -> p kj h", p=128)
        w2lo_v = w2lo_d.ap().rearrange("(kj p) h -> p kj h", p=128)
        for kj in range(J1):
            nc.scalar.dma_start(out=w2H[:, kj * H2:(kj + 1) * H2],
                                in_=w2h_v[:, kj, :])
        for kj in range(J1):
            nc.sync.dma_start(out=w2LO[:, kj * H2:(kj + 1) * H2],
                              in_=w2lo_v[:, kj, :])
        w3hT = wsb.tile([128, J2 * O], F16)
        nc.sync.dma_start(
            out=w3hT.rearrange("p (kj o) -> p kj o", kj=J2),
            in_=w3h_d.ap().rearrange("(kj p) o -> p kj o", p=128))
        b2cT = wsb.tile([128, J2], F32)
        rs3T = wsb.tile([1, O], F16)
        onesT = wsb.tile([1, 128], F16)
        nc.sync.dma_start(out=b2cT, in_=b2c_d.ap())
        nc.sync.dma_start(out=rs3T, in_=rs3h_d.ap())
        nc.sync.dma_start(out=onesT, in_=ones16_d.ap())
        nbias = wsb.tile([128, 1], F32)
        nc.vector.memset(nbias, -1.0)

        # ---- scan scratch -------------------------------------------------
        # LIF state kept NEGATED (nmem = -mem), written in place over cur:
        #   A: tmp   = (nmem * beta) - cur          [= -(beta*mem + cur)]
        #   B: nmem' = (nmem is_lt -1) + tmp        [= -(tmp_ref - spike)]
        # fp32 RNE is sign-symmetric => bit-identical to the reference.
        tmp1 = wsb.tile([128, J1 * BL], F32)
        tmp2 = wsb.tile([128, J2 * BL], F32)
        zf1 = wsb.tile([128, J1 * BL], F32)
        zf2 = wsb.tile([128, J2 * BL], F32)
        nc.vector.memset(zf1, 0.0)
        nc.vector.memset(zf2, 0.0)
        # PE warm-up: trip the HAM clock gate before chunk 0's matmuls.
        nwarm = opts.get("pe_warmup", 48)
        if nwarm:
            wpt = pp3.tile([64, 64], F32, tag="pp3", name="warmpt")
            for _ in range(nwarm):
                nc.tensor.matmul(wpt, lhsT=zf1[:, 0:64], rhs=zf1,
                                 start=True, stop=True)

        toff = [0]
        for s in sched:
            toff.append(toff[-1] + s)

        GG = opts.get("grp", 16)  # spike-extraction group size (steps)

        # ---- per-stage emitters (software-pipelined below) ---------------
        def emit_mm1(c, first=False):
            """fp16 3-pass cur1, one PSUM group per j-tile.
            cur1 scan layout: col = t*64 + j*8 + b   (h1 = j*128 + p)."""
            tcsz = sched[c]
            ntb = tcsz * BL
            tb0 = toff[c] * BL
            if first:
                xs = xT0
            else:
                xs = [xt_pool.tile([128, KI * ntb], F16, tag=f"x{i}",
                                   name=f"xc{c}_{i}")
                      for i in range(3)]
                for i in range(3):
                    xqs[i].dma_start(
                        out=xs[i].rearrange("p (io tb) -> p io tb", io=KI),
                        in_=x_dvs[i][:, :, tb0:tb0 + ntb])
            cur1 = cur1_pool.tile([128, tcsz * J1 * BL], F32, tag="cur1")
            cur1_v = cur1.rearrange("p (t j b) -> p t j b", t=tcsz, j=J1, b=BL)
            for j in range(J1):
                pt = pp1.tile([128, ntb], F32, tag="pp1")
                i_mm = 0
                for io in range(KI):
                    for xi in (0, 1):  # xh, xl vs stationary w1h
                        nc.tensor.matmul(
                            pt,
                            lhsT=w1H[:, io * H1 + j * 128:
                                     io * H1 + (j + 1) * 128],
                            rhs=xs[xi][:, io * ntb:(io + 1) * ntb],
                            start=(i_mm == 0), stop=False)
                        i_mm += 1
                for io in range(KI):  # xh>>10 vs stationary w1l<<10
                    nc.tensor.matmul(
                        pt,
                        lhsT=w1L[:, io * H1 + j * 128:
                                 io * H1 + (j + 1) * 128],
                        rhs=xs[2][:, io * ntb:(io + 1) * ntb],
                        start=False, stop=(io == KI - 1))
                nc.scalar.activation(
                    cur1_v[:, :, j, :],
                    pt.rearrange("p (t b) -> p t b", b=BL), ACTF.Copy)
            return cur1

        def scan1_state(c, cur1):
            """Allocate chunk-c scan-1 outputs."""
            tcsz = sched[c]
            W = J1 * BL
            spk1 = spk1_pool.tile([128, tcsz * W], F16, tag="spk1")
            s12 = s12_pool.tile([128, tcsz * W], F8E5, tag="s12")
            return dict(c=c, t=0, tcsz=tcsz, W=W, cur=cur1, spk=spk1,
                        s12=s12)

        def scan1_step(st, prev, solo=False):
            """One LIF step of layer 1 (A+B on DVE); extraction at group
            boundaries.  prev = (nmem tile of chunk c-1, its tcsz).
            solo=True splits the ops into two independent j-half streams so
            the DVE pipelines them (no interleave partner available)."""
            t, W, cur1 = st["t"], st["W"], st["cur"]
            cs = cur1[:, t * W:(t + 1) * W]
            if t == 0:
                nprev = zf1 if prev is None else \
                    prev[0][:, (prev[1] - 1) * W: prev[1] * W]
            else:
                nprev = cur1[:, (t - 1) * W: t * W]
            if solo:
                h = W // 2
                nc.vector.scalar_tensor_tensor(
                    tmp1[:, 0:h], nprev[:, 0:h], BETA1, cs[:, 0:h],
                    ALU.mult, ALU.subtract)
                nc.vector.scalar_tensor_tensor(
                    tmp1[:, h:W], nprev[:, h:W], BETA1, cs[:, h:W],
                    ALU.mult, ALU.subtract)
                nc.vector.scalar_tensor_tensor(
                    cs[:, 0:h], nprev[:, 0:h], -1.0, tmp1[:, 0:h],
                    ALU.is_lt, ALU.add)
                nc.vector.scalar_tensor_tensor(
                    cs[:, h:W], nprev[:, h:W], -1.0, tmp1[:, h:W],
                    ALU.is_lt, ALU.add)
            else:
                nc.vector.scalar_tensor_tensor(
                    tmp1, nprev, BETA1, cs, ALU.mult, ALU.subtract)
                nc.vector.scalar_tensor_tensor(
                    cs, nprev, -1.0, tmp1, ALU.is_lt, ALU.add)
            if t % GG == GG - 1 or t == st["tcsz"] - 1:
                g0 = (t // GG) * GG
                so = st["spk"][:, g0 * W:(t + 1) * W]
                si = cur1[:, g0 * W:(t + 1) * W]
                if ext1 == "act":
                    nc.scalar.activation(so, si, ACTF.Sign,
                                         bias=nbias, scale=-1.0)
                elif ext1 == "pool":
                    nc.gpsimd.tensor_scalar(so, si, -1.0, None, ALU.is_lt)
                else:
                    nc.vector.tensor_scalar(so, si, -1.0, None, ALU.is_lt)
                nc.scalar.activation(
                    st["s12"][:, g0 * W:(t + 1) * W], so, ACTF.Copy,
                    scale=float(2.0 ** -12))
            st["t"] = t + 1

        def emit_mm2(c, spk1, s12):
            """fp16-hi (8 matmuls) + e4m3xe5m2 DoubleRow lo (4 matmuls at
            2x rate) per j-tile, one PSUM group."""
            tcsz = sched[c]
            ntb = tcsz * BL
            spk1_v = spk1.rearrange("p (t j b) -> p t j b", t=tcsz, j=J1, b=BL)
            s12_v = s12.rearrange("p (t j b) -> p t j b", t=tcsz, j=J1, b=BL)
            w2lo_v = w2LO.rearrange("p (kj h) -> p kj h", kj=J1)
            cur2 = cur2_pool.tile([128, tcsz * J2 * BL], F32, tag="cur2")
            cur2_v = cur2.rearrange("p (t j b) -> p t j b", t=tcsz, j=J2, b=BL)
            for j in range(J2):
                pt = pp2.tile([128, ntb], F32, tag="pp2")
                for kj in range(J1):
                    nc.tensor.matmul(
                        pt,
                        lhsT=w2H[:, kj * H2 + j * 128: kj * H2 + (j + 1) * 128],
                        rhs=spk1_v[:, :, kj, :],
                        start=(kj == 0), stop=False)
                for a in range(J1 // 2):
                    nc.tensor.matmul(
                        pt,
                        lhsT=w2lo_v[:, 2 * a:2 * a + 2,
                                    j * 128:(j + 1) * 128],
                        rhs=s12_v[:, :, 2 * a:2 * a + 2, :].rearrange(
                            "p t kt b -> p kt t b"),
                        start=False, stop=(a == J1 // 2 - 1),
                        perf_mode=DR, skip_group_check=True)
                if ext1 == "act":
                    # cur2 = 0.5*psum + rowsum(w2_eff)/2  (spikes are +-1)
                    nc.scalar.activation(
                        cur2_v[:, :, j, :],
                        pt.rearrange("p (t b) -> p t b", b=BL), ACTF.Identity,
                        bias=b2cT[:, j:j + 1], scale=0.5)
                else:
                    nc.scalar.activation(
                        cur2_v[:, :, j, :],
                        pt.rearrange("p (t b) -> p t b", b=BL), ACTF.Copy)
            return cur2

        def scan2_state(c, cur2):
            tcsz = sched[c]
            W = J2 * BL
            spk2 = spk2_pool.tile([128, tcsz * W], F16, tag="spk2")
            return dict(c=c, t=0, tcsz=tcsz, W=W, cur=cur2, spk=spk2)

        def scan2_step(st, prev, solo=False):
            """One LIF step of layer 2; spk2 written j-major
            (col = j*ntb + t*8 + b) for matmul-3 stationary reads."""
            t, W, cur2 = st["t"], st["W"], st["cur"]
            tcsz = st["tcsz"]
            cs = cur2[:, t * W:(t + 1) * W]
            if t == 0:
                nprev = zf2 if prev is None else \
                    prev[0][:, (prev[1] - 1) * W: prev[1] * W]
            else:
                nprev = cur2[:, (t - 1) * W: t * W]
            if solo:
                h = W // 2
                nc.vector.scalar_tensor_tensor(
                    tmp2[:, 0:h], nprev[:, 0:h], BETA2, cs[:, 0:h],
                    ALU.mult, ALU.subtract)
                nc.vector.scalar_tensor_tensor(
                    tmp2[:, h:W], nprev[:, h:W], BETA2, cs[:, h:W],
                    ALU.mult, ALU.subtract)
                nc.vector.scalar_tensor_tensor(
                    cs[:, 0:h], nprev[:, 0:h], -1.0, tmp2[:, 0:h],
                    ALU.is_lt, ALU.add)
                nc.vector.scalar_tensor_tensor(
                    cs[:, h:W], nprev[:, h:W], -1.0, tmp2[:, h:W],
                    ALU.is_lt, ALU.add)
            else:
                nc.vector.scalar_tensor_tensor(
                    tmp2, nprev, BETA2, cs, ALU.mult, ALU.subtract)
                nc.vector.scalar_tensor_tensor(
                    cs, nprev, -1.0, tmp2, ALU.is_lt, ALU.add)
            if t % GG == GG - 1 or t == tcsz - 1:
                g0 = (t // GG) * GG
                spk2_tv = st["spk"].rearrange("p (j t b) -> p t j b",
                                              j=J2, t=tcsz, b=BL)
                cur2_v = cur2.rearrange("p (t j b) -> p t j b",
                                        t=tcsz, j=J2, b=BL)
                so = spk2_tv[:, g0:t + 1, :, :]
                si = cur2_v[:, g0:t + 1, :, :]
                if ext2 == "act":
                    nc.scalar.activation(so, si, ACTF.Sign,
                                         bias=nbias, scale=-1.0)
                elif ext2 == "pool":
                    nc.gpsimd.tensor_scalar(so, si, -1.0, None, ALU.is_lt)
                else:
                    nc.vector.tensor_scalar(so, si, -1.0, None, ALU.is_lt)
            st["t"] = t + 1

        def emit_mm3(c, spk2):
            """fp16 single pass: out[tb, o] = spk2 @ w3^T."""
            tcsz = sched[c]
            ntb = tcsz * BL
            tb0 = toff[c] * BL
            for m0 in range(0, ntb, 128):
                msz = min(128, ntb - m0)
                pt = pp3.tile([msz, O], F32, tag="pp3")
                for kj in range(J2):
                    nc.tensor.matmul(
                        pt,
                        lhsT=spk2[:, kj * ntb + m0: kj * ntb + m0 + msz],
                        rhs=w3hT[:, kj * O:(kj + 1) * O],
                        start=(kj == 0), stop=(kj == J2 - 1 and ext2 != "act"))
                if ext2 == "act":
                    # +rowsum(w3) row: out = (w3 @ s± + rs3) then *0.5
                    nc.tensor.matmul(pt, lhsT=onesT[:, 0:msz], rhs=rs3T,
                                     start=False, stop=True)
                osb = out_pool.tile([msz, O], F32, tag="osb")
                nc.scalar.activation(osb, pt, ACTF.Copy,
                                     scale=(0.5 if ext2 == "act" else 1.0))
                r0 = tb0 + m0
                nc.sync.dma_start(out=y[r0:r0 + msz, :], in_=osb)

        # ---- software-pipelined emission ---------------------------------
        hwloop = opts.get("hwloop", 1)
        loop_cm = tc_ctx.For_i(0, hwloop, 1) if hwloop > 1 else None
        if loop_cm is not None:
            loop_cm.__enter__()
        for _r in range(opts.get("repeat", 1)):
            cur1 = emit_mm1(0, first=(_r == 0 and hwloop == 1))
            st1 = scan1_state(0, cur1)
            nm1 = prev1 = None
            while st1["t"] < st1["tcsz"]:
                scan1_step(st1, None, solo=True)
            nm1 = (st1["cur"], st1["tcsz"])
            spk2 = {}
            nm2 = None
            st2 = None
            for c in range(nch):
                if c + 1 < nch:
                    cur1 = emit_mm1(c + 1)
                cur2 = emit_mm2(c, st1["spk"], st1["s12"])
                st2 = scan2_state(c, cur2)
                if c + 1 < nch:
                    st1n = scan1_state(c + 1, cur1)
                    # op-by-op interleave of the two independent chains:
                    # the DVE pipelines alternating-stream instructions
                    # (~144ns/op vs ~341ns serial, HW-measured).
                    while st1n["t"] < st1n["tcsz"] or st2["t"] < st2["tcsz"]:
                        a_live = st1n["t"] < st1n["tcsz"]
                        b_live = st2["t"] < st2["tcsz"]
                        if a_live:
                            scan1_step(st1n, nm1, solo=not b_live)
                        if b_live:
                            scan2_step(st2, nm2, solo=not a_live)
                    nm1 = (st1n["cur"], st1n["tcsz"])
                    st1 = st1n
                else:
                    while st2["t"] < st2["tcsz"]:
                        scan2_step(st2, nm2, solo=True)
                spk2[c] = st2["spk"]
                nm2 = (st2["cur"], st2["tcsz"])
            # All matmul-3 tiles run in the epilogue (PE drain filler).
            for cc in range(nch):
                emit_mm3(cc, spk2.pop(cc))
        if loop_cm is not None:
            loop_cm.__exit__(None, None, None)


def prep_inputs(x, w1, w2, w3, n_t=T):
    """Host-side layout prep shared by kernel() and tests."""
    x = np.asarray(x, dtype=np.float32)
    w1 = np.asarray(w1, dtype=np.float32)
    w2 = np.asarray(w2, dtype=np.float32)
    w3 = np.asarray(w3, dtype=np.float32)
    SC = np.float32(2.0 ** 10)
    w1t = np.ascontiguousarray(w1.T)                       # [I, H1] f32
    w1hs = w1t.astype(np.float16)
    w1ls = ((w1t - w1hs.astype(np.float32)) * SC).astype(np.float16)
    w2t = np.ascontiguousarray(w2.T)                       # [H1, H2] f32
    w2hs = w2t.astype(np.float16)
    w2lo = ((w2t - w2hs.astype(np.float32)) * np.float32(2.0 ** 12)) \
        .astype(ml_dtypes.float8_e4m3)
    w3t = np.ascontiguousarray(w3.T)                       # [H2, O] f32
    w3hs = w3t.astype(np.float16)
    w2_eff = w2hs.astype(np.float64) + w2lo.astype(np.float64) * 2.0 ** -12
    rs2 = w2_eff.sum(axis=0)                               # [H2]
    b2c = (rs2 * 0.5).astype(np.float32).reshape(J2, 128).T  # [128, J2]
    rs3 = w3hs.astype(np.float64).sum(axis=0)              # [O]
    common = {
        "w1h": w1hs,
        "w1l10": w1ls,
        "w2h": w2hs,
        "w2lo": w2lo,
        "w3h": w3hs,
        "b2c": np.ascontiguousarray(b2c),
        "rs3h": rs3.astype(np.float16).reshape(1, O),
        "ones16": np.ones((1, 128), np.float16),
    }
    xcores = []
    for cid in range(NCORES):
        xs = x[:, cid * BL:(cid + 1) * BL, :].reshape(n_t * BL, I)
        xT = np.ascontiguousarray(xs.T)                    # [I, n_t*BL] f32
        xh = xT.astype(np.float16)
        xl = (xT - xh.astype(np.float32)).astype(np.float16)
        xh10 = (xh.astype(np.float32) / SC).astype(np.float16)
        xcores.append({"xh": xh, "xl": xl, "xh10": xh10})
    return xcores, common


_NC_CACHE = {}


def _get_nc():
    if "nc" not in _NC_CACHE:
        _NC_CACHE["nc"] = build()
    return _NC_CACHE["nc"]


def kernel(x, w1, b1, w2, b2, w3, b3, **_unused):
    """Full inputs in, full output out. b1/b2/b3 are zeros in this problem
    (asserted) -- the device program skips the bias adds."""
    assert not np.any(np.asarray(b1)) and not np.any(np.asarray(b2)) \
        and not np.any(np.asarray(b3)), "nonzero biases unsupported"

    nc = _get_nc()
    xcores, common = prep_inputs(x, w1, w2, w3)
    in_maps = [{**xcores[cid], **common} for cid in range(NCORES)]
    res = run_bass_kernel_spmd(nc, in_maps, list(range(NCORES)))
    outs = [r["y"].reshape(T, BL, O) for r in res.results]
    return np.concatenate(outs, axis=1)


if __name__ == "__main__":
    nc = build()
    print("built OK")


# revision 12
# speedup vs baseline: 1.1587x; 1.0890x over previous
"""Trainium2 Bass kernel for a 3-layer spiking net (snntorch-style Leaky/LIF).

Math (per timestep t, eval mode):
    cur1 = x_t @ w1.T + b1
    mem1 = 0.9*mem1 + cur1 - (mem1_prev > 1)        # reset-by-subtract
    spk1 = (mem1 > 1)
    cur2 = spk1 @ w2.T + b2
    mem2 = 0.85*mem2 + cur2 - (mem2_prev > 1)
    spk2 = (mem2 > 1)
    out_t = spk2 @ w3.T + b3

Strategy (v2):
  - Data-parallel over batch: B=64 -> 8 cores x 8.
  - Matmuls batched over T in chunks; only the elementwise LIF updates are
    sequential.  The serial chains (2 scalar_tensor_tensor ops per step per
    layer, fp32, bit-exact vs the reference via the negated-membrane trick)
    run on the DVE; NOTHING else runs on the DVE, so the chain streams
    back-to-back (~127ns/op, 1024 ops ~ 130us).
  - Spike extraction (batched is_lt every 16 steps) runs on the Pool
    engine (gpsimd), which is otherwise idle -- it no longer interrupts
    the DVE chain.  spk tiles are fp16 {0,1} (exact).
  - Matmul 1: fp16 3-pass as before (x hi/lo + w1 lo with 2^10 shifts),
    known band ~2.7e-7.  82us of PE.
  - Matmul 2: fp16-hi + fp8-lo: cur2 = w2h^T s + (w2lo8^T s12) where
    w2h = fp16(w2), w2lo8 = e4m3((w2 - w2h) * 2^12) via a DoubleRow
    (2x-rate) fp8 matmul against s12 = e5m2(spk * 2^-12) (exact: powers
    of two).  Effective w2 residual ~2^-21, same as the old bf16 hi/lo
    2-pass, at 10/16 the PE cycles (68us).  Probe-verified exact on HW.
  - Matmul 3: fp16 w3 single pass (output error ~1e-4, was 1.5e-3 bf16).
  - s12 tiles are derived from the fp16 spikes by ACT copies with scale
    2^-12 (exact, probe-verified).
  - Pipeline: MM1(c+1) -> MM2(c) on PE; scan1(c+1) before scan2(c) on
    DVE; MM3 in the epilogue to fill the PE drain window.
"""

import sys

for _p in ("/opt/trn_rl_repo", "/root/.axon_site/_ro/pypackages"):
    if _p not in sys.path:
        sys.path.insert(0, _p)

import ml_dtypes
import numpy as np

import concourse.bass as bass
import concourse.mybir as mybir
from concourse import bacc, tile
from concourse.bass_utils import run_bass_kernel_spmd

F32 = mybir.dt.float32
F16 = mybir.dt.float16
BF16 = mybir.dt.bfloat16
F8E4 = mybir.dt.float8e4
F8E5 = mybir.dt.float8e5
ALU = mybir.AluOpType
ACTF = mybir.ActivationFunctionType
DR = mybir.MatmulPerfMode.DoubleRow

# Problem shape (hardcoded; harness runs kernel.py standalone).
T, B, I, H1, H2, O = 256, 64, 512, 1024, 1024, 256
NCORES = 8
BL = B // NCORES          # batch per core
BETA1, BETA2 = 0.9, 0.85
KI = I // 128             # K-tiles for matmul 1 (4)
J1 = H1 // 128            # M-tiles for layer 1 (8)
J2 = H2 // 128            # M-tiles for layer 2 (8)


def build(n_t=T, sched=None, trace_sim=False, opts=None):
    """Build the per-core SPMD program. Identical on all cores."""
    if sched is None:
        sched = (opts or {}).get("sched") or [32, 64, 64, 64, 32]
    assert sum(sched) == n_t and all(s % 8 == 0 for s in sched)
    nc = bacc.Bacc("TRN2", target_bir_lowering=False, debug=False)

    xh = nc.declare_dram_parameter("xh", [I, n_t * BL], F16, isOutput=False)
    xl = nc.declare_dram_parameter("xl", [I, n_t * BL], F16, isOutput=False)
    w1h = nc.declare_dram_parameter("w1h", [I, H1], F16, isOutput=False)
    w1l10 = nc.declare_dram_parameter("w1l10", [I, H1], F16, isOutput=False)
    w2h = nc.declare_dram_parameter("w2h", [H1, H2], F16, isOutput=False)
    w2lo = nc.declare_dram_parameter("w2lo", [H1, H2], F8E4, isOutput=False)
    w3h = nc.declare_dram_parameter("w3h", [H2, O], F16, isOutput=False)
    b2c = nc.declare_dram_parameter("b2c", [128, J2], F32, isOutput=False)
    rs3h = nc.declare_dram_parameter("rs3h", [1, O], F16, isOutput=False)
    ones16 = nc.declare_dram_parameter("ones16", [1, 128], F16,
                                       isOutput=False)
    y = nc.declare_dram_parameter("y", [n_t * BL, O], F32, isOutput=True)

    with tile.TileContext(nc, trace_sim=trace_sim) as tc_ctx:
        _body(nc, tc_ctx, (xh, xl), (w1h, w1l10), w2h, w2lo, w3h,
              (b2c, rs3h, ones16), y, n_t, sched, opts or {})
    nc.compile()
    return nc


def _body(nc, tc_ctx, x_ds, w1_ds, w2h_d, w2lo_d, w3h_d, corr_ds, y,
          n_t, sched, opts):
    b2c_d, rs3h_d, ones16_d = corr_ds
    ext = opts.get("ext", "dve")
    no_scan = opts.get("no_scan", False)
    no_mm1 = opts.get("no_mm1", False)
    no_mm2 = opts.get("no_mm2", False)
    no_mm3 = opts.get("no_mm3", False)
    no_xdma = opts.get("no_xdma", False)
    no_ydma = opts.get("no_ydma", False)
    ext1 = opts.get("ext1", ext)
    ext2 = opts.get("ext2", ext)
    nch = len(sched)
    ntb0 = sched[0] * BL
    import contextlib

    ctx = contextlib.ExitStack()
    with ctx:
        cb = opts.get("cur_bufs", 2)
        pb1, pb2, pb3 = opts.get("psum_bufs", (3, 3, 2))
        wsb = ctx.enter_context(tc_ctx.tile_pool(name="wsb", bufs=1))
        xt_pool = ctx.enter_context(
            tc_ctx.tile_pool(name="xt", bufs=opts.get("xt_bufs", 2)))
        cur1_pool = ctx.enter_context(tc_ctx.tile_pool(name="cur1", bufs=cb))
        spk1_pool = ctx.enter_context(tc_ctx.tile_pool(name="spk1", bufs=2))
        s12_pool = ctx.enter_context(tc_ctx.tile_pool(name="s12", bufs=2))
        cur2_pool = ctx.enter_context(tc_ctx.tile_pool(name="cur2", bufs=cb))
        spk2_pool = ctx.enter_context(tc_ctx.tile_pool(name="spk2",
                                                       bufs=len(sched)))
        out_pool = ctx.enter_context(tc_ctx.tile_pool(name="outp", bufs=2))
        pp1 = ctx.enter_context(tc_ctx.tile_pool(name="pp1", bufs=pb1,
                                                 space="PSUM"))
        pp2 = ctx.enter_context(tc_ctx.tile_pool(name="pp2", bufs=pb2,
                                                 space="PSUM"))
        pp3 = ctx.enter_context(tc_ctx.tile_pool(name="pp3", bufs=pb3,
                                                 space="PSUM"))

        # ---- weight loads (pre-transposed on host) ----------------------
        x_dvs = [xd.ap().rearrange("(io p) tb -> p io tb", p=128)
                 for xd in x_ds]
        w1H = wsb.tile([128, KI * H1], F16)
        w1L = wsb.tile([128, KI * H1], F16)
        w1h_v = w1_ds[0].ap().rearrange("(io p) h -> p io h", p=128)
        w1l_v = w1_ds[1].ap().rearrange("(io p) h -> p io h", p=128)
        xT0 = [xt_pool.tile([128, KI * ntb0], F16, tag=f"x{i}",
                            name=f"xT0_{i}")
               for i in range(3)]
        xqs = (nc.sync, nc.scalar)
        # fill: x chunk-0 slabs first (one DMA each), then w1 io-slices
        # (needed by MM1(0)), then the rest as single merged DMAs.
        for i in range(2):
            xqs[i].dma_start(
                out=xT0[i].rearrange("p (io tb) -> p io tb", io=KI),
                in_=x_dvs[i][:, :, 0:ntb0])
        for io in range(KI):
            nc.scalar.dma_start(out=w1H[:, io * H1:(io + 1) * H1],
                                in_=w1h_v[:, io, :])
            nc.sync.dma_start(out=w1L[:, io * H1:(io + 1) * H1],
                              in_=w1l_v[:, io, :])
        w2H = wsb.tile([128, J1 * H2], F16)
        w2LO = wsb.tile([128, J1 * H2], F8E4)
        w2h_v = w2h_d.ap().rearrange("(kj p) h -> p kj h", p=128)
        w2lo_v = w2lo_d.ap().rearrange("(kj p) h -> p kj h", p=128)
        nc.gpsimd.dma_start(out=w2H.rearrange("p (kj h) -> p kj h", kj=J1),
                            in_=w2h_v)
        nc.gpsimd.dma_start(out=w2LO.rearrange("p (kj h) -> p kj h", kj=J1),
                            in_=w2lo_v)
        w3hT = wsb.tile([128, J2 * O], F16)
        nc.sync.dma_start(
            out=w3hT.rearrange("p (kj o) -> p kj o", kj=J2),
            in_=w3h_d.ap().rearrange("(kj p) o -> p kj o", p=128))
        b2cT = wsb.tile([128, J2], F32)
        rs3T = wsb.tile([1, O], F16)
        onesT = wsb.tile([1, 128], F16)
        nc.sync.dma_start(out=b2cT, in_=b2c_d.ap())
        nc.sync.dma_start(out=rs3T, in_=rs3h_d.ap())
        nc.sync.dma_start(out=onesT, in_=ones16_d.ap())
        nbias = wsb.tile([128, 1], F32)
        nc.vector.memset(nbias, -1.0)

        # ---- scan scratch -------------------------------------------------
        # LIF state kept NEGATED (nmem = -mem), written in place over cur:
        #   A: tmp   = (nmem * beta) - cur          [= -(beta*mem + cur)]
        #   B: nmem' = (nmem is_lt -1) + tmp        [= -(tmp_ref - spike)]
        # fp32 RNE is sign-symmetric => bit-identical to the reference.
        tmp1 = wsb.tile([128, J1 * BL], F32)
        tmp2 = wsb.tile([128, J2 * BL], F32)
        zf1 = wsb.tile([128, J1 * BL], F32)
        zf2 = wsb.tile([128, J2 * BL], F32)
        nc.vector.memset(zf1, 0.0)
        nc.vector.memset(zf2, 0.0)
        # PE warm-up: trip the HAM clock gate before chunk 0's matmuls.
        nwarm = opts.get("pe_warmup", 48)
        if nwarm:
            wpt = pp3.tile([64, 64], F32, tag="pp3", name="warmpt")
            for _ in range(nwarm):
                nc.tensor.matmul(wpt, lhsT=zf1[:, 0:64], rhs=zf1,
                                 start=True, stop=True)

        toff = [0]
        for s in sched:
            toff.append(toff[-1] + s)

        GG = opts.get("grp", 16)  # spike-extraction group size (steps)

        # ---- per-stage emitters (software-pipelined below) ---------------
        def emit_mm1(c, first=False):
            """fp16 3-pass cur1, one PSUM group per j-tile.
            cur1 scan layout: col = t*64 + j*8 + b   (h1 = j*128 + p)."""
            tcsz = sched[c]
            ntb = tcsz * BL
            tb0 = toff[c] * BL
            if first:
                xs = xT0
            else:
                xs = [xt_pool.tile([128, KI * ntb], F16, tag=f"x{i}",
                                   name=f"xc{c}_{i}")
                      for i in range(3)]
                if not no_xdma:
                    for i in range(2):
                        xqs[i].dma_start(
                            out=xs[i].rearrange("p (io tb) -> p io tb", io=KI),
                            in_=x_dvs[i][:, :, tb0:tb0 + ntb])
            # xh10 = xh * 2^-10 derived on ACT (same fp16 RNE as host)
            nc.scalar.activation(xs[2], xs[0], ACTF.Copy,
                                 scale=float(2.0 ** -10))
            cur1 = cur1_pool.tile([128, tcsz * J1 * BL], F32, tag="cur1")
            cur1_v = cur1.rearrange("p (t j b) -> p t j b", t=tcsz, j=J1, b=BL)
            if no_mm1:
                return cur1
            for j in range(J1):
                pt = pp1.tile([128, ntb], F32, tag="pp1")
                i_mm = 0
                for io in range(KI):
                    for xi in (0, 1):  # xh, xl vs stationary w1h
                        nc.tensor.matmul(
                            pt,
                            lhsT=w1H[:, io * H1 + j * 128:
                                     io * H1 + (j + 1) * 128],
                            rhs=xs[xi][:, io * ntb:(io + 1) * ntb],
                            start=(i_mm == 0), stop=False)
                        i_mm += 1
                for io in range(KI):  # xh>>10 vs stationary w1l<<10
                    nc.tensor.matmul(
                        pt,
                        lhsT=w1L[:, io * H1 + j * 128:
                                 io * H1 + (j + 1) * 128],
                        rhs=xs[2][:, io * ntb:(io + 1) * ntb],
                        start=False, stop=(io == KI - 1))
                nc.scalar.activation(
                    cur1_v[:, :, j, :],
                    pt.rearrange("p (t b) -> p t b", b=BL), ACTF.Copy)
            return cur1

        def scan1_state(c, cur1):
            """Allocate chunk-c scan-1 outputs."""
            tcsz = sched[c]
            W = J1 * BL
            spk1 = spk1_pool.tile([128, tcsz * W], F16, tag="spk1")
            s12 = s12_pool.tile([128, tcsz * W], F8E5, tag="s12")
            return dict(c=c, t=0, tcsz=tcsz, W=W, cur=cur1, spk=spk1,
                        s12=s12)

        def scan1_step(st, prev, solo=False):
            """One LIF step of layer 1 (A+B on DVE); extraction at group
            boundaries.  prev = (nmem tile of chunk c-1, its tcsz).
            solo=True splits the ops into two independent j-half streams so
            the DVE pipelines them (no interleave partner available)."""
            t, W, cur1 = st["t"], st["W"], st["cur"]
            cs = cur1[:, t * W:(t + 1) * W]
            if t == 0:
                nprev = zf1 if prev is None else \
                    prev[0][:, (prev[1] - 1) * W: prev[1] * W]
            else:
                nprev = cur1[:, (t - 1) * W: t * W]
            if no_scan:
                pass
            elif solo:
                h = W // 2
                nc.vector.scalar_tensor_tensor(
                    tmp1[:, 0:h], nprev[:, 0:h], BETA1, cs[:, 0:h],
                    ALU.mult, ALU.subtract)
                nc.vector.scalar_tensor_tensor(
                    tmp1[:, h:W], nprev[:, h:W], BETA1, cs[:, h:W],
                    ALU.mult, ALU.subtract)
                nc.vector.scalar_tensor_tensor(
                    cs[:, 0:h], nprev[:, 0:h], -1.0, tmp1[:, 0:h],
                    ALU.is_lt, ALU.add)
                nc.vector.scalar_tensor_tensor(
                    cs[:, h:W], nprev[:, h:W], -1.0, tmp1[:, h:W],
                    ALU.is_lt, ALU.add)
            else:
                nc.vector.scalar_tensor_tensor(
                    tmp1, nprev, BETA1, cs, ALU.mult, ALU.subtract)
                nc.vector.scalar_tensor_tensor(
                    cs, nprev, -1.0, tmp1, ALU.is_lt, ALU.add)
            if t % GG == GG - 1 or t == st["tcsz"] - 1:
                g0 = (t // GG) * GG
                so = st["spk"][:, g0 * W:(t + 1) * W]
                si = cur1[:, g0 * W:(t + 1) * W]
                if ext1 == "act":
                    nc.scalar.activation(so, si, ACTF.Sign,
                                         bias=nbias, scale=-1.0)
                elif ext1 == "pool":
                    nc.gpsimd.tensor_scalar(so, si, -1.0, None, ALU.is_lt)
                else:
                    nc.vector.tensor_scalar(so, si, -1.0, None, ALU.is_lt)
                nc.scalar.activation(
                    st["s12"][:, g0 * W:(t + 1) * W], so, ACTF.Copy,
                    scale=float(2.0 ** -12))
            st["t"] = t + 1

        def emit_mm2(c, spk1, s12):
            """fp16-hi (8 matmuls) + e4m3xe5m2 DoubleRow lo (4 matmuls at
            2x rate) per j-tile, one PSUM group."""
            tcsz = sched[c]
            ntb = tcsz * BL
            spk1_v = spk1.rearrange("p (t j b) -> p t j b", t=tcsz, j=J1, b=BL)
            s12_v = s12.rearrange("p (t j b) -> p t j b", t=tcsz, j=J1, b=BL)
            w2lo_v = w2LO.rearrange("p (kj h) -> p kj h", kj=J1)
            cur2 = cur2_pool.tile([128, tcsz * J2 * BL], F32, tag="cur2")
            cur2_v = cur2.rearrange("p (t j b) -> p t j b", t=tcsz, j=J2, b=BL)
            if no_mm2:
                return cur2
            for j in range(J2):
                pt = pp2.tile([128, ntb], F32, tag="pp2")
                for kj in range(J1):
                    nc.tensor.matmul(
                        pt,
                        lhsT=w2H[:, kj * H2 + j * 128: kj * H2 + (j + 1) * 128],
                        rhs=spk1_v[:, :, kj, :],
                        start=(kj == 0), stop=False)
                for a in range(J1 // 2):
                    nc.tensor.matmul(
                        pt,
                        lhsT=w2lo_v[:, 2 * a:2 * a + 2,
                                    j * 128:(j + 1) * 128],
                        rhs=s12_v[:, :, 2 * a:2 * a + 2, :].rearrange(
                            "p t kt b -> p kt t b"),
                        start=False, stop=(a == J1 // 2 - 1),
                        perf_mode=DR, skip_group_check=True)
                if ext1 == "act":
                    # cur2 = 0.5*psum + rowsum(w2_eff)/2  (spikes are +-1)
                    nc.scalar.activation(
                        cur2_v[:, :, j, :],
                        pt.rearrange("p (t b) -> p t b", b=BL), ACTF.Identity,
                        bias=b2cT[:, j:j + 1], scale=0.5)
                else:
                    nc.scalar.activation(
                        cur2_v[:, :, j, :],
                        pt.rearrange("p (t b) -> p t b", b=BL), ACTF.Copy)
            return cur2

        def scan2_state(c, cur2):
            tcsz = sched[c]
            W = J2 * BL
            spk2 = spk2_pool.tile([128, tcsz * W], F16, tag="spk2")
            return dict(c=c, t=0, tcsz=tcsz, W=W, cur=cur2, spk=spk2)

        def scan2_step(st, prev, solo=False):
            """One LIF step of layer 2; spk2 written j-major
            (col = j*ntb + t*8 + b) for matmul-3 stationary reads."""
            t, W, cur2 = st["t"], st["W"], st["cur"]
            tcsz = st["tcsz"]
            cs = cur2[:, t * W:(t + 1) * W]
            if t == 0:
                nprev = zf2 if prev is None else \
                    prev[0][:, (prev[1] - 1) * W: prev[1] * W]
            else:
                nprev = cur2[:, (t - 1) * W: t * W]
            if no_scan:
                pass
            elif solo:
                h = W // 2
                nc.vector.scalar_tensor_tensor(
                    tmp2[:, 0:h], nprev[:, 0:h], BETA2, cs[:, 0:h],
                    ALU.mult, ALU.subtract)
                nc.vector.scalar_tensor_tensor(
                    tmp2[:, h:W], nprev[:, h:W], BETA2, cs[:, h:W],
                    ALU.mult, ALU.subtract)
                nc.vector.scalar_tensor_tensor(
                    cs[:, 0:h], nprev[:, 0:h], -1.0, tmp2[:, 0:h],
                    ALU.is_lt, ALU.add)
                nc.vector.scalar_tensor_tensor(
                    cs[:, h:W], nprev[:, h:W], -1.0, tmp2[:, h:W],
                    ALU.is_lt, ALU.add)
            else:
                nc.vector.scalar_tensor_tensor(
                    tmp2, nprev, BETA2, cs, ALU.mult, ALU.subtract)
                nc.vector.scalar_tensor_tensor(
                    cs, nprev, -1.0, tmp2, ALU.is_lt, ALU.add)
            if t % GG == GG - 1 or t == tcsz - 1:
                g0 = (t // GG) * GG
                spk2_tv = st["spk"].rearrange("p (j t b) -> p t j b",
                                              j=J2, t=tcsz, b=BL)
                cur2_v = cur2.rearrange("p (t j b) -> p t j b",
                                        t=tcsz, j=J2, b=BL)
                so = spk2_tv[:, g0:t + 1, :, :]
                si = cur2_v[:, g0:t + 1, :, :]
                if ext2 == "act":
                    nc.scalar.activation(so, si, ACTF.Sign,
                                         bias=nbias, scale=-1.0)
                elif ext2 == "pool":
                    nc.gpsimd.tensor_scalar(so, si, -1.0, None, ALU.is_lt)
                else:
                    nc.vector.tensor_scalar(so, si, -1.0, None, ALU.is_lt)
            st["t"] = t + 1

        def emit_mm3(c, spk2):
            """fp16 single pass: out[tb, o] = spk2 @ w3^T.
            All m-tiles of the chunk collect into one SBUF slab; a single
            DMA per chunk writes y (DMA latency ~2.7us per op dominates
            many small stores)."""
            if no_mm3:
                return
            tcsz = sched[c]
            ntb = tcsz * BL
            tb0 = toff[c] * BL
            nmt = ntb // 128
            osb = out_pool.tile([128, nmt * O], F32, tag="osb")
            for mt in range(nmt):
                m0 = mt * 128
                pt = pp3.tile([128, O], F32, tag="pp3")
                for kj in range(J2):
                    nc.tensor.matmul(
                        pt,
                        lhsT=spk2[:, kj * ntb + m0: kj * ntb + m0 + 128],
                        rhs=w3hT[:, kj * O:(kj + 1) * O],
                        start=(kj == 0), stop=(kj == J2 - 1 and ext2 != "act"))
                if ext2 == "act":
                    nc.tensor.matmul(pt, lhsT=onesT[:, 0:128], rhs=rs3T,
                                     start=False, stop=True)
                nc.scalar.activation(osb[:, mt * O:(mt + 1) * O], pt,
                                     ACTF.Copy,
                                     scale=(0.5 if ext2 == "act" else 1.0))
            if not no_ydma:
                nc.gpsimd.dma_start(
                    out=y[tb0:tb0 + ntb, :].rearrange(
                        "(mt p) o -> p mt o", p=128),
                    in_=osb.rearrange("p (mt o) -> p mt o", mt=nmt))

        # ---- software-pipelined emission ---------------------------------
        hwloop = opts.get("hwloop", 1)
        loop_cm = tc_ctx.For_i(0, hwloop, 1) if hwloop > 1 else None
        if loop_cm is not None:
            loop_cm.__enter__()
        for _r in range(opts.get("repeat", 1)):
            cur1 = emit_mm1(0, first=(_r == 0 and hwloop == 1))
            st1 = scan1_state(0, cur1)
            nm1 = prev1 = None
            while st1["t"] < st1["tcsz"]:
                scan1_step(st1, None, solo=True)
            nm1 = (st1["cur"], st1["tcsz"])
            spk2 = {}
            nm2 = None
            st2 = None
            for c in range(nch):
                if c + 1 < nch:
                    cur1 = emit_mm1(c + 1)
                cur2 = emit_mm2(c, st1["spk"], st1["s12"])
                st2 = scan2_state(c, cur2)
                if c + 1 < nch:
                    st1n = scan1_state(c + 1, cur1)
                    # op-by-op interleave of the two independent chains:
                    # the DVE pipelines alternating-stream instructions
                    # (~144ns/op vs ~341ns serial, HW-measured).
                    while st1n["t"] < st1n["tcsz"] or st2["t"] < st2["tcsz"]:
                        a_live = st1n["t"] < st1n["tcsz"]
                        b_live = st2["t"] < st2["tcsz"]
                        if a_live:
                            scan1_step(st1n, nm1, solo=not b_live)
                        if b_live:
                            scan2_step(st2, nm2, solo=not a_live)
                    nm1 = (st1n["cur"], st1n["tcsz"])
                    st1 = st1n
                else:
                    while st2["t"] < st2["tcsz"]:
                        scan2_step(st2, nm2, solo=True)
                spk2[c] = st2["spk"]
                nm2 = (st2["cur"], st2["tcsz"])
            # All matmul-3 tiles run in the epilogue (PE drain filler).
            for cc in range(nch):
                emit_mm3(cc, spk2.pop(cc))
        if loop_cm is not None:
            loop_cm.__exit__(None, None, None)


def prep_inputs(x, w1, w2, w3, n_t=T):
    """Host-side layout prep shared by kernel() and tests."""
    x = np.asarray(x, dtype=np.float32)
    w1 = np.asarray(w1, dtype=np.float32)
    w2 = np.asarray(w2, dtype=np.float32)
    w3 = np.asarray(w3, dtype=np.float32)
    SC = np.float32(2.0 ** 10)
    w1t = np.ascontiguousarray(w1.T)                       # [I, H1] f32
    w1hs = w1t.astype(np.float16)
    w1ls = ((w1t - w1hs.astype(np.float32)) * SC).astype(np.float16)
    w2t = np.ascontiguousarray(w2.T)                       # [H1, H2] f32
    w2hs = w2t.astype(np.float16)
    w2lo = ((w2t - w2hs.astype(np.float32)) * np.float32(2.0 ** 12)) \
        .astype(ml_dtypes.float8_e4m3)
    w3t = np.ascontiguousarray(w3.T)                       # [H2, O] f32
    w3hs = w3t.astype(np.float16)
    w2_eff = w2hs.astype(np.float64) + w2lo.astype(np.float64) * 2.0 ** -12
    rs2 = w2_eff.sum(axis=0)                               # [H2]
    b2c = (rs2 * 0.5).astype(np.float32).reshape(J2, 128).T  # [128, J2]
    rs3 = w3hs.astype(np.float64).sum(axis=0)              # [O]
    common = {
        "w1h": w1hs,
        "w1l10": w1ls,
        "w2h": w2hs,
        "w2lo": w2lo,
        "w3h": w3hs,
        "b2c": np.ascontiguousarray(b2c),
        "rs3h": rs3.astype(np.float16).reshape(1, O),
        "ones16": np.ones((1, 128), np.float16),
    }
    xcores = []
    for cid in range(NCORES):
        xs = x[:, cid * BL:(cid + 1) * BL, :].reshape(n_t * BL, I)
        xT = np.ascontiguousarray(xs.T)                    # [I, n_t*BL] f32
        xh = xT.astype(np.float16)
        xl = (xT - xh.astype(np.float32)).astype(np.float16)
        xcores.append({"xh": xh, "xl": xl})
    return xcores, common


_NC_CACHE = {}


def _get_nc():
    if "nc" not in _NC_CACHE:
        _NC_CACHE["nc"] = build()
    return _NC_CACHE["nc"]


def kernel(x, w1, b1, w2, b2, w3, b3, **_unused):
    """Full inputs in, full output out. b1/b2/b3 are zeros in this problem
    (asserted) -- the device program skips the bias adds."""
    assert not np.any(np.asarray(b1)) and not np.any(np.asarray(b2)) \
        and not np.any(np.asarray(b3)), "nonzero biases unsupported"

    nc = _get_nc()
    xcores, common = prep_inputs(x, w1, w2, w3)
    in_maps = [{**xcores[cid], **common} for cid in range(NCORES)]
    res = run_bass_kernel_spmd(nc, in_maps, list(range(NCORES)))
    outs = [r["y"].reshape(T, BL, O) for r in res.results]
    return np.concatenate(outs, axis=1)


if __name__ == "__main__":
    nc = build()
    print("built OK")


# revision 14
# speedup vs baseline: 1.3058x; 1.1270x over previous
"""Trainium2 Bass kernel for a 3-layer spiking net (snntorch-style Leaky/LIF).

Math (per timestep t, eval mode):
    cur1 = x_t @ w1.T + b1
    mem1 = 0.9*mem1 + cur1 - (mem1_prev > 1)        # reset-by-subtract
    spk1 = (mem1 > 1)
    cur2 = spk1 @ w2.T + b2
    mem2 = 0.85*mem2 + cur2 - (mem2_prev > 1)
    spk2 = (mem2 > 1)
    out_t = spk2 @ w3.T + b3

Strategy (v2):
  - Data-parallel over batch: B=64 -> 8 cores x 8.
  - Matmuls batched over T in chunks; only the elementwise LIF updates are
    sequential.  The serial chains (2 scalar_tensor_tensor ops per step per
    layer, fp32, bit-exact vs the reference via the negated-membrane trick)
    run on the DVE; NOTHING else runs on the DVE, so the chain streams
    back-to-back (~127ns/op, 1024 ops ~ 130us).
  - Spike extraction (batched is_lt every 16 steps) runs on the Pool
    engine (gpsimd), which is otherwise idle -- it no longer interrupts
    the DVE chain.  spk tiles are fp16 {0,1} (exact).
  - Matmul 1: fp16 3-pass as before (x hi/lo + w1 lo with 2^10 shifts),
    known band ~2.7e-7.  82us of PE.
  - Matmul 2: fp16-hi + fp8-lo: cur2 = w2h^T s + (w2lo8^T s12) where
    w2h = fp16(w2), w2lo8 = e4m3((w2 - w2h) * 2^12) via a DoubleRow
    (2x-rate) fp8 matmul against s12 = e5m2(spk * 2^-12) (exact: powers
    of two).  Effective w2 residual ~2^-21, same as the old bf16 hi/lo
    2-pass, at 10/16 the PE cycles (68us).  Probe-verified exact on HW.
  - Matmul 3: fp16 w3 single pass (output error ~1e-4, was 1.5e-3 bf16).
  - s12 tiles are derived from the fp16 spikes by ACT copies with scale
    2^-12 (exact, probe-verified).
  - Pipeline: MM1(c+1) -> MM2(c) on PE; scan1(c+1) before scan2(c) on
    DVE; MM3 in the epilogue to fill the PE drain window.
"""

import sys

for _p in ("/opt/trn_rl_repo", "/root/.axon_site/_ro/pypackages"):
    if _p not in sys.path:
        sys.path.insert(0, _p)

import ml_dtypes
import numpy as np

import concourse.bass as bass
import concourse.mybir as mybir
from concourse import bacc, tile
from concourse.bass_utils import run_bass_kernel_spmd

F32 = mybir.dt.float32
F16 = mybir.dt.float16
BF16 = mybir.dt.bfloat16
F8E4 = mybir.dt.float8e4
F8E5 = mybir.dt.float8e5
ALU = mybir.AluOpType
ACTF = mybir.ActivationFunctionType
DR = mybir.MatmulPerfMode.DoubleRow

# Problem shape (hardcoded; harness runs kernel.py standalone).
T, B, I, H1, H2, O = 256, 64, 512, 1024, 1024, 256
NCORES = 8
BL = B // NCORES          # batch per core
BETA1, BETA2 = 0.9, 0.85
KI = I // 128             # K-tiles for matmul 1 (4)
J1 = H1 // 128            # M-tiles for layer 1 (8)
J2 = H2 // 128            # M-tiles for layer 2 (8)


def build(n_t=T, sched=None, trace_sim=False, opts=None):
    """Build the per-core SPMD program. Identical on all cores."""
    if sched is None:
        sched = (opts or {}).get("sched") or [32] * 8
    assert sum(sched) == n_t and all(s % 8 == 0 for s in sched)
    nc = bacc.Bacc("TRN2", target_bir_lowering=False, debug=False)

    xh = nc.declare_dram_parameter("xh", [I, n_t * BL], F16, isOutput=False)
    xl = nc.declare_dram_parameter("xl", [I, n_t * BL], F16, isOutput=False)
    w1h = nc.declare_dram_parameter("w1h", [I, H1], F16, isOutput=False)
    w1l10 = nc.declare_dram_parameter("w1l10", [I, H1], F16, isOutput=False)
    w2h = nc.declare_dram_parameter("w2h", [H1, H2], F16, isOutput=False)
    w2lo = nc.declare_dram_parameter("w2lo", [H1, H2], F8E4, isOutput=False)
    w3h = nc.declare_dram_parameter("w3h", [H2, O], F16, isOutput=False)
    b2c = nc.declare_dram_parameter("b2c", [128, J2], F32, isOutput=False)
    rs3h = nc.declare_dram_parameter("rs3h", [1, O], F16, isOutput=False)
    ones16 = nc.declare_dram_parameter("ones16", [1, 128], F16,
                                       isOutput=False)
    y = nc.declare_dram_parameter("y", [n_t * BL, O], F32, isOutput=True)

    with tile.TileContext(nc, trace_sim=trace_sim) as tc_ctx:
        _body(nc, tc_ctx, (xh, xl), (w1h, w1l10), w2h, w2lo, w3h,
              (b2c, rs3h, ones16), y, n_t, sched, opts or {})
    nc.compile()
    return nc


def _body(nc, tc_ctx, x_ds, w1_ds, w2h_d, w2lo_d, w3h_d, corr_ds, y,
          n_t, sched, opts):
    b2c_d, rs3h_d, ones16_d = corr_ds
    ext = opts.get("ext", "dve")
    no_scan = opts.get("no_scan", False)
    no_mm1 = opts.get("no_mm1", False)
    no_mm2 = opts.get("no_mm2", False)
    no_mm3 = opts.get("no_mm3", False)
    no_xdma = opts.get("no_xdma", False)
    no_ydma = opts.get("no_ydma", False)
    ext1 = opts.get("ext1", ext)
    ext2 = opts.get("ext2", ext)
    nch = len(sched)
    ntb0 = sched[0] * BL
    import contextlib

    ctx = contextlib.ExitStack()
    with ctx:
        cb = opts.get("cur_bufs", 2)
        pb1, pb2, pb3 = opts.get("psum_bufs", (3, 3, 2))
        wsb = ctx.enter_context(tc_ctx.tile_pool(name="wsb", bufs=1))
        xt_pool = ctx.enter_context(
            tc_ctx.tile_pool(name="xt", bufs=opts.get("xt_bufs", 2)))
        cur1_pool = ctx.enter_context(tc_ctx.tile_pool(name="cur1", bufs=opts.get("c1_bufs", 3)))
        spk1_pool = ctx.enter_context(tc_ctx.tile_pool(name="spk1", bufs=3))
        s12_pool = ctx.enter_context(tc_ctx.tile_pool(name="s12", bufs=3))
        cur2_pool = ctx.enter_context(tc_ctx.tile_pool(name="cur2", bufs=opts.get("c2_bufs", 3)))
        spk2_pool = ctx.enter_context(tc_ctx.tile_pool(name="spk2",
                                                       bufs=len(sched)))
        out_pool = ctx.enter_context(tc_ctx.tile_pool(name="outp", bufs=2))
        pp1 = ctx.enter_context(tc_ctx.tile_pool(name="pp1", bufs=pb1,
                                                 space="PSUM"))
        pp2 = ctx.enter_context(tc_ctx.tile_pool(name="pp2", bufs=pb2,
                                                 space="PSUM"))
        pp3 = ctx.enter_context(tc_ctx.tile_pool(name="pp3", bufs=pb3,
                                                 space="PSUM"))

        # ---- weight loads (pre-transposed on host) ----------------------
        x_dvs = [xd.ap().rearrange("(io p) tb -> p io tb", p=128)
                 for xd in x_ds]
        w1H = wsb.tile([128, KI * H1], F16)
        w1L = wsb.tile([128, KI * H1], F16)
        w1h_v = w1_ds[0].ap().rearrange("(io p) h -> p io h", p=128)
        w1l_v = w1_ds[1].ap().rearrange("(io p) h -> p io h", p=128)
        xT0 = [xt_pool.tile([128, KI * ntb0], F16, tag=f"x{i}",
                            name=f"xT0_{i}")
               for i in range(3)]
        xqs = (nc.sync, nc.scalar)
        # fill: x chunk-0 slabs first (one DMA each), then w1 io-slices
        # (needed by MM1(0)), then the rest as single merged DMAs.
        for i in range(2):
            xqs[i].dma_start(
                out=xT0[i].rearrange("p (io tb) -> p io tb", io=KI),
                in_=x_dvs[i][:, :, 0:ntb0])
        for io in range(KI):
            nc.scalar.dma_start(out=w1H[:, io * H1:(io + 1) * H1],
                                in_=w1h_v[:, io, :])
            nc.sync.dma_start(out=w1L[:, io * H1:(io + 1) * H1],
                              in_=w1l_v[:, io, :])
        w2H = wsb.tile([128, J1 * H2], F16)
        w2LO = wsb.tile([128, J1 * H2], F8E4)
        w2h_v = w2h_d.ap().rearrange("(kj p) h -> p kj h", p=128)
        w2lo_v = w2lo_d.ap().rearrange("(kj p) h -> p kj h", p=128)
        nc.gpsimd.dma_start(out=w2H.rearrange("p (kj h) -> p kj h", kj=J1),
                            in_=w2h_v)
        nc.gpsimd.dma_start(out=w2LO.rearrange("p (kj h) -> p kj h", kj=J1),
                            in_=w2lo_v)
        w3hT = wsb.tile([128, J2 * O], F16)
        nc.sync.dma_start(
            out=w3hT.rearrange("p (kj o) -> p kj o", kj=J2),
            in_=w3h_d.ap().rearrange("(kj p) o -> p kj o", p=128))
        b2cT = wsb.tile([128, J2], F32)
        rs3T = wsb.tile([1, O], F16)
        onesT = wsb.tile([1, 128], F16)
        nc.sync.dma_start(out=b2cT, in_=b2c_d.ap())
        nc.sync.dma_start(out=rs3T, in_=rs3h_d.ap())
        nc.sync.dma_start(out=onesT, in_=ones16_d.ap())
        nbias = wsb.tile([128, 1], F32)
        nc.vector.memset(nbias, -1.0)

        # ---- scan scratch -------------------------------------------------
        # LIF state kept NEGATED (nmem = -mem), written in place over cur:
        #   A: tmp   = (nmem * beta) - cur          [= -(beta*mem + cur)]
        #   B: nmem' = (nmem is_lt -1) + tmp        [= -(tmp_ref - spike)]
        # fp32 RNE is sign-symmetric => bit-identical to the reference.
        tmp1 = wsb.tile([128, J1 * BL], F32)
        tmp2 = wsb.tile([128, J2 * BL], F32)
        zf1 = wsb.tile([128, J1 * BL], F32)
        zf2 = wsb.tile([128, J2 * BL], F32)
        nc.vector.memset(zf1, 0.0)
        nc.vector.memset(zf2, 0.0)
        # PE warm-up: trip the HAM clock gate before chunk 0's matmuls.
        nwarm = opts.get("pe_warmup", 48)
        if nwarm:
            wpt = pp3.tile([64, 64], F32, tag="pp3", name="warmpt")
            for _ in range(nwarm):
                nc.tensor.matmul(wpt, lhsT=zf1[:, 0:64], rhs=zf1,
                                 start=True, stop=True)

        toff = [0]
        for s in sched:
            toff.append(toff[-1] + s)

        GG = opts.get("grp", 16)  # spike-extraction group size (steps)

        # ---- per-stage emitters (software-pipelined below) ---------------
        def emit_mm1(c, first=False):
            """fp16 3-pass cur1, one PSUM group per j-tile.
            cur1 scan layout: col = t*64 + j*8 + b   (h1 = j*128 + p)."""
            tcsz = sched[c]
            ntb = tcsz * BL
            tb0 = toff[c] * BL
            if first:
                xs = xT0
            else:
                xs = [xt_pool.tile([128, KI * ntb], F16, tag=f"x{i}",
                                   name=f"xc{c}_{i}")
                      for i in range(3)]
                if not no_xdma:
                    for i in range(2):
                        xqs[i].dma_start(
                            out=xs[i].rearrange("p (io tb) -> p io tb", io=KI),
                            in_=x_dvs[i][:, :, tb0:tb0 + ntb])
            # xh10 = xh * 2^-10 derived on ACT (same fp16 RNE as host)
            nc.scalar.activation(xs[2], xs[0], ACTF.Copy,
                                 scale=float(2.0 ** -10))
            cur1 = cur1_pool.tile([128, tcsz * J1 * BL], F32, tag="cur1")
            cur1_v = cur1.rearrange("p (t j b) -> p t j b", t=tcsz, j=J1, b=BL)
            if no_mm1:
                return cur1
            for j in range(J1):
                pt = pp1.tile([128, ntb], F32, tag="pp1")
                i_mm = 0
                for io in range(KI):
                    for xi in (0, 1):  # xh, xl vs stationary w1h
                        nc.tensor.matmul(
                            pt,
                            lhsT=w1H[:, io * H1 + j * 128:
                                     io * H1 + (j + 1) * 128],
                            rhs=xs[xi][:, io * ntb:(io + 1) * ntb],
                            start=(i_mm == 0), stop=False)
                        i_mm += 1
                for io in range(KI):  # xh>>10 vs stationary w1l<<10
                    nc.tensor.matmul(
                        pt,
                        lhsT=w1L[:, io * H1 + j * 128:
                                 io * H1 + (j + 1) * 128],
                        rhs=xs[2][:, io * ntb:(io + 1) * ntb],
                        start=False, stop=(io == KI - 1))
                nc.scalar.activation(
                    cur1_v[:, :, j, :],
                    pt.rearrange("p (t b) -> p t b", b=BL), ACTF.Copy)
            return cur1

        def scan1_state(c, cur1):
            """Allocate chunk-c scan-1 outputs."""
            tcsz = sched[c]
            W = J1 * BL
            spk1 = spk1_pool.tile([128, tcsz * W], F16, tag="spk1")
            s12 = s12_pool.tile([128, tcsz * W], F8E5, tag="s12")
            return dict(c=c, t=0, tcsz=tcsz, W=W, cur=cur1, spk=spk1,
                        s12=s12)

        def scan1_step(st, prev, solo=False):
            """One LIF step of layer 1 (A+B on DVE); extraction at group
            boundaries.  prev = (nmem tile of chunk c-1, its tcsz).
            solo=True splits the ops into two independent j-half streams so
            the DVE pipelines them (no interleave partner available)."""
            t, W, cur1 = st["t"], st["W"], st["cur"]
            cs = cur1[:, t * W:(t + 1) * W]
            if t == 0:
                nprev = zf1 if prev is None else \
                    prev[0][:, (prev[1] - 1) * W: prev[1] * W]
            else:
                nprev = cur1[:, (t - 1) * W: t * W]
            if no_scan:
                pass
            elif solo:
                h = W // 2
                nc.vector.scalar_tensor_tensor(
                    tmp1[:, 0:h], nprev[:, 0:h], BETA1, cs[:, 0:h],
                    ALU.mult, ALU.subtract)
                nc.vector.scalar_tensor_tensor(
                    tmp1[:, h:W], nprev[:, h:W], BETA1, cs[:, h:W],
                    ALU.mult, ALU.subtract)
                nc.vector.scalar_tensor_tensor(
                    cs[:, 0:h], nprev[:, 0:h], -1.0, tmp1[:, 0:h],
                    ALU.is_lt, ALU.add)
                nc.vector.scalar_tensor_tensor(
                    cs[:, h:W], nprev[:, h:W], -1.0, tmp1[:, h:W],
                    ALU.is_lt, ALU.add)
            else:
                nc.vector.scalar_tensor_tensor(
                    tmp1, nprev, BETA1, cs, ALU.mult, ALU.subtract)
                nc.vector.scalar_tensor_tensor(
                    cs, nprev, -1.0, tmp1, ALU.is_lt, ALU.add)
            if t % GG == GG - 1 or t == st["tcsz"] - 1:
                g0 = (t // GG) * GG
                so = st["spk"][:, g0 * W:(t + 1) * W]
                si = cur1[:, g0 * W:(t + 1) * W]
                if ext1 == "act":
                    nc.scalar.activation(so, si, ACTF.Sign,
                                         bias=nbias, scale=-1.0)
                elif ext1 == "pool":
                    nc.gpsimd.tensor_scalar(so, si, -1.0, None, ALU.is_lt)
                else:
                    nc.vector.tensor_scalar(so, si, -1.0, None, ALU.is_lt)
                nc.scalar.activation(
                    st["s12"][:, g0 * W:(t + 1) * W], so, ACTF.Copy,
                    scale=float(2.0 ** -12))
            st["t"] = t + 1

        def emit_mm2(c, spk1, s12):
            """fp16-hi (8 matmuls) + e4m3xe5m2 DoubleRow lo (4 matmuls at
            2x rate) per j-tile, one PSUM group."""
            tcsz = sched[c]
            ntb = tcsz * BL
            spk1_v = spk1.rearrange("p (t j b) -> p t j b", t=tcsz, j=J1, b=BL)
            s12_v = s12.rearrange("p (t j b) -> p t j b", t=tcsz, j=J1, b=BL)
            w2lo_v = w2LO.rearrange("p (kj h) -> p kj h", kj=J1)
            cur2 = cur2_pool.tile([128, tcsz * J2 * BL], F32, tag="cur2")
            cur2_v = cur2.rearrange("p (t j b) -> p t j b", t=tcsz, j=J2, b=BL)
            if no_mm2:
                return cur2
            for j in range(J2):
                pt = pp2.tile([128, ntb], F32, tag="pp2")
                for kj in range(J1):
                    nc.tensor.matmul(
                        pt,
                        lhsT=w2H[:, kj * H2 + j * 128: kj * H2 + (j + 1) * 128],
                        rhs=spk1_v[:, :, kj, :],
                        start=(kj == 0), stop=False)
                for a in range(J1 // 2):
                    nc.tensor.matmul(
                        pt,
                        lhsT=w2lo_v[:, 2 * a:2 * a + 2,
                                    j * 128:(j + 1) * 128],
                        rhs=s12_v[:, :, 2 * a:2 * a + 2, :].rearrange(
                            "p t kt b -> p kt t b"),
                        start=False, stop=(a == J1 // 2 - 1),
                        perf_mode=DR, skip_group_check=True)
                if ext1 == "act":
                    # cur2 = 0.5*psum + rowsum(w2_eff)/2  (spikes are +-1)
                    nc.scalar.activation(
                        cur2_v[:, :, j, :],
                        pt.rearrange("p (t b) -> p t b", b=BL), ACTF.Identity,
                        bias=b2cT[:, j:j + 1], scale=0.5)
                else:
                    nc.scalar.activation(
                        cur2_v[:, :, j, :],
                        pt.rearrange("p (t b) -> p t b", b=BL), ACTF.Copy)
            return cur2

        def scan2_state(c, cur2):
            tcsz = sched[c]
            W = J2 * BL
            spk2 = spk2_pool.tile([128, tcsz * W], F16, tag="spk2")
            return dict(c=c, t=0, tcsz=tcsz, W=W, cur=cur2, spk=spk2)

        def scan2_step(st, prev, solo=False):
            """One LIF step of layer 2; spk2 written j-major
            (col = j*ntb + t*8 + b) for matmul-3 stationary reads."""
            t, W, cur2 = st["t"], st["W"], st["cur"]
            tcsz = st["tcsz"]
            cs = cur2[:, t * W:(t + 1) * W]
            if t == 0:
                nprev = zf2 if prev is None else \
                    prev[0][:, (prev[1] - 1) * W: prev[1] * W]
            else:
                nprev = cur2[:, (t - 1) * W: t * W]
            if no_scan:
                pass
            elif solo:
                h = W // 2
                nc.vector.scalar_tensor_tensor(
                    tmp2[:, 0:h], nprev[:, 0:h], BETA2, cs[:, 0:h],
                    ALU.mult, ALU.subtract)
                nc.vector.scalar_tensor_tensor(
                    tmp2[:, h:W], nprev[:, h:W], BETA2, cs[:, h:W],
                    ALU.mult, ALU.subtract)
                nc.vector.scalar_tensor_tensor(
                    cs[:, 0:h], nprev[:, 0:h], -1.0, tmp2[:, 0:h],
                    ALU.is_lt, ALU.add)
                nc.vector.scalar_tensor_tensor(
                    cs[:, h:W], nprev[:, h:W], -1.0, tmp2[:, h:W],
                    ALU.is_lt, ALU.add)
            else:
                nc.vector.scalar_tensor_tensor(
                    tmp2, nprev, BETA2, cs, ALU.mult, ALU.subtract)
                nc.vector.scalar_tensor_tensor(
                    cs, nprev, -1.0, tmp2, ALU.is_lt, ALU.add)
            if t % GG == GG - 1 or t == tcsz - 1:
                g0 = (t // GG) * GG
                spk2_tv = st["spk"].rearrange("p (j t b) -> p t j b",
                                              j=J2, t=tcsz, b=BL)
                cur2_v = cur2.rearrange("p (t j b) -> p t j b",
                                        t=tcsz, j=J2, b=BL)
                so = spk2_tv[:, g0:t + 1, :, :]
                si = cur2_v[:, g0:t + 1, :, :]
                if ext2 == "act":
                    nc.scalar.activation(so, si, ACTF.Sign,
                                         bias=nbias, scale=-1.0)
                elif ext2 == "pool":
                    nc.gpsimd.tensor_scalar(so, si, -1.0, None, ALU.is_lt)
                else:
                    nc.vector.tensor_scalar(so, si, -1.0, None, ALU.is_lt)
            st["t"] = t + 1

        def emit_mm3(c, spk2):
            """fp16 single pass: out[tb, o] = spk2 @ w3^T.
            All m-tiles of the chunk collect into one SBUF slab; a single
            DMA per chunk writes y (DMA latency ~2.7us per op dominates
            many small stores)."""
            if no_mm3:
                return
            tcsz = sched[c]
            ntb = tcsz * BL
            tb0 = toff[c] * BL
            nmt = ntb // 128
            osb = out_pool.tile([128, nmt * O], F32, tag="osb")
            for mt in range(nmt):
                m0 = mt * 128
                pt = pp3.tile([128, O], F32, tag="pp3")
                for kj in range(J2):
                    nc.tensor.matmul(
                        pt,
                        lhsT=spk2[:, kj * ntb + m0: kj * ntb + m0 + 128],
                        rhs=w3hT[:, kj * O:(kj + 1) * O],
                        start=(kj == 0), stop=(kj == J2 - 1 and ext2 != "act"))
                if ext2 == "act":
                    nc.tensor.matmul(pt, lhsT=onesT[:, 0:128], rhs=rs3T,
                                     start=False, stop=True)
                nc.scalar.activation(osb[:, mt * O:(mt + 1) * O], pt,
                                     ACTF.Copy,
                                     scale=(0.5 if ext2 == "act" else 1.0))
            if not no_ydma:
                nc.gpsimd.dma_start(
                    out=y[tb0:tb0 + ntb, :].rearrange(
                        "(mt p) o -> p mt o", p=128),
                    in_=osb.rearrange("p (mt o) -> p mt o", mt=nmt))

        # ---- software-pipelined emission ---------------------------------
        hwloop = opts.get("hwloop", 1)
        loop_cm = tc_ctx.For_i(0, hwloop, 1) if hwloop > 1 else None
        if loop_cm is not None:
            loop_cm.__enter__()
        for _r in range(opts.get("repeat", 1)):
            deep = opts.get("deep", True)
            if deep:
                # Deep pipeline: phase c emits MM1(c+2), MM2(c),
                # scan1(c+1) x scan2(c-1).  Every phase's inputs were
                # produced a full phase earlier, so neither engine waits
                # intra-phase.
                cur1s = {0: emit_mm1(0, first=(_r == 0 and hwloop == 1))}
                st1 = scan1_state(0, cur1s[0])
                if 1 < nch:
                    cur1s[1] = emit_mm1(1)
                while st1["t"] < st1["tcsz"]:
                    scan1_step(st1, None, solo=True)
                nm1 = (st1["cur"], st1["tcsz"])
                sp1 = {0: st1}
                spk2 = {}
                nm2 = None
                st2s = {}
                for c in range(nch):
                    if c + 2 < nch:
                        cur1s[c + 2] = emit_mm1(c + 2)
                    cur2 = emit_mm2(c, sp1[c]["spk"], sp1[c]["s12"])
                    st2s[c] = scan2_state(c, cur2)
                    st1n = scan1_state(c + 1, cur1s[c + 1]) \
                        if c + 1 < nch else None
                    st2p = st2s.get(c - 1)
                    while (st1n is not None and st1n["t"] < st1n["tcsz"]) \
                            or (st2p is not None and st2p["t"] < st2p["tcsz"]):
                        a_live = st1n is not None and st1n["t"] < st1n["tcsz"]
                        b_live = st2p is not None and st2p["t"] < st2p["tcsz"]
                        if a_live:
                            scan1_step(st1n, nm1, solo=not b_live)
                        if b_live:
                            scan2_step(st2p, nm2 if st2p["t"] > 0 or c - 1 == 0
                                       else nm2, solo=not a_live)
                    if st2p is not None:
                        spk2[c - 1] = st2p["spk"]
                        nm2 = (st2p["cur"], st2p["tcsz"])
                    if st1n is not None:
                        nm1 = (st1n["cur"], st1n["tcsz"])
                        sp1[c + 1] = st1n
                # drain: scan2(nch-1)
                st2p = st2s[nch - 1]
                while st2p["t"] < st2p["tcsz"]:
                    scan2_step(st2p, nm2, solo=True)
                spk2[nch - 1] = st2p["spk"]
                for cc in range(nch):
                    emit_mm3(cc, spk2.pop(cc))
            else:
                cur1 = emit_mm1(0, first=(_r == 0 and hwloop == 1))
                st1 = scan1_state(0, cur1)
                nm1 = prev1 = None
                while st1["t"] < st1["tcsz"]:
                    scan1_step(st1, None, solo=True)
                nm1 = (st1["cur"], st1["tcsz"])
                spk2 = {}
                nm2 = None
                st2 = None
                for c in range(nch):
                    if c + 1 < nch:
                        cur1 = emit_mm1(c + 1)
                    cur2 = emit_mm2(c, st1["spk"], st1["s12"])
                    st2 = scan2_state(c, cur2)
                    if c + 1 < nch:
                        st1n = scan1_state(c + 1, cur1)
                        while st1n["t"] < st1n["tcsz"] or st2["t"] < st2["tcsz"]:
                            a_live = st1n["t"] < st1n["tcsz"]
                            b_live = st2["t"] < st2["tcsz"]
                            if a_live:
                                scan1_step(st1n, nm1, solo=not b_live)
                            if b_live:
                                scan2_step(st2, nm2, solo=not a_live)
                        nm1 = (st1n["cur"], st1n["tcsz"])
                        st1 = st1n
                    else:
                        while st2["t"] < st2["tcsz"]:
                            scan2_step(st2, nm2, solo=True)
                    spk2[c] = st2["spk"]
                    nm2 = (st2["cur"], st2["tcsz"])
                for cc in range(nch):
                    emit_mm3(cc, spk2.pop(cc))
        if loop_cm is not None:
            loop_cm.__exit__(None, None, None)


def prep_inputs(x, w1, w2, w3, n_t=T):
    """Host-side layout prep shared by kernel() and tests."""
    x = np.asarray(x, dtype=np.float32)
    w1 = np.asarray(w1, dtype=np.float32)
    w2 = np.asarray(w2, dtype=np.float32)
    w3 = np.asarray(w3, dtype=np.float32)
    SC = np.float32(2.0 ** 10)
    w1t = np.ascontiguousarray(w1.T)                       # [I, H1] f32
    w1hs = w1t.astype(np.float16)
    w1ls = ((w1t - w1hs.astype(np.float32)) * SC).astype(np.float16)
    w2t = np.ascontiguousarray(w2.T)                       # [H1, H2] f32
    w2hs = w2t.astype(np.float16)
    w2lo = ((w2t - w2hs.astype(np.float32)) * np.float32(2.0 ** 12)) \
        .astype(ml_dtypes.float8_e4m3)
    w3t = np.ascontiguousarray(w3.T)                       # [H2, O] f32
    w3hs = w3t.astype(np.float16)
    w2_eff = w2hs.astype(np.float64) + w2lo.astype(np.float64) * 2.0 ** -12
    rs2 = w2_eff.sum(axis=0)                               # [H2]
    b2c = (rs2 * 0.5).astype(np.float32).reshape(J2, 128).T  # [128, J2]
    rs3 = w3hs.astype(np.float64).sum(axis=0)              # [O]
    common = {
        "w1h": w1hs,
        "w1l10": w1ls,
        "w2h": w2hs,
        "w2lo": w2lo,
        "w3h": w3hs,
        "b2c": np.ascontiguousarray(b2c),
        "rs3h": rs3.astype(np.float16).reshape(1, O),
        "ones16": np.ones((1, 128), np.float16),
    }
    xcores = []
    for cid in range(NCORES):
        xs = x[:, cid * BL:(cid + 1) * BL, :].reshape(n_t * BL, I)
        xT = np.ascontiguousarray(xs.T)                    # [I, n_t*BL] f32
        xh = xT.astype(np.float16)
        xl = (xT - xh.astype(np.float32)).astype(np.float16)
        xcores.append({"xh": xh, "xl": xl})
    return xcores, common


_NC_CACHE = {}


def _get_nc():
    if "nc" not in _NC_CACHE:
        _NC_CACHE["nc"] = build()
    return _NC_CACHE["nc"]


def kernel(x, w1, b1, w2, b2, w3, b3, **_unused):
    """Full inputs in, full output out. b1/b2/b3 are zeros in this problem
    (asserted) -- the device program skips the bias adds."""
    assert not np.any(np.asarray(b1)) and not np.any(np.asarray(b2)) \
        and not np.any(np.asarray(b3)), "nonzero biases unsupported"

    nc = _get_nc()
    xcores, common = prep_inputs(x, w1, w2, w3)
    in_maps = [{**xcores[cid], **common} for cid in range(NCORES)]
    res = run_bass_kernel_spmd(nc, in_maps, list(range(NCORES)))
    outs = [r["y"].reshape(T, BL, O) for r in res.results]
    return np.concatenate(outs, axis=1)


if __name__ == "__main__":
    nc = build()
    print("built OK")
